# revision 5
# baseline (speedup 1.0000x reference)
"""GCN layer on 8 TRN2 cores — scatter-add architecture.

Sharding: cores own contiguous SRC ranges (edge/data parallel per the
sharding hint). Each core computes scaled messages X'[src]=feat*ci for its
~200k out-edges and dma_scatter_add's them into a per-core partial
aggregate table in DRAM (fp16 pair-rows of 256 B; even dst nodes at cols
0:48, odd at 64:112). A ReduceScatter (add) over the 8 partial tables
gives each core the summed aggregates for its 6250-dst slice; the tail
(cj scaling, zero-in-degree fallback, 48x48 linear + relu) runs locally.

Message expansion is free for the first WPOS edges per (src, dst-parity):
scatter instruction r reads the X' SBUF tile directly as its token payload
(token position == local node id).  Remaining edges are host-pre-gathered
raw feature rows, scaled by rsqrt(out_deg) on device; rows receiving many
such edges are pre-summed on the vector engine (8-way groups) so each
leftover stream stays large.

dma_scatter_add loses duplicate target rows *within* one instruction
(parallel read-modify-write against a snapshot), so every instruction's
rows are made unique by construction: positional streams evict in-
instruction duplicates to the dense path; dense streams take the k-th
edge of each (row, parity) bucket; group-reduction emits one token per
row.  Instructions on the same parity view are serialized by the
framework's WAW dependency, which makes cross-instruction accumulation
exact; the two parity views are byte-disjoint so their chains overlap.
"""

import numpy as np

N = 50000
E = 1600000
D = 48
NCORES = 8
NPC = 6250              # real nodes per core
NLOC = 6272             # padded local nodes (49 * 128)
BLOCKS = 49
PAIRS_L = 3125          # real local pairs
ROWS_PC = 3200          # table rows per core (incl. junk rows)
TROWS = ROWS_PC * NCORES
TCOLS = 128             # 256 B rows: E@0:48  O@64:112
WPOS = 14               # positional ranks per (node, dst-parity)
NRAW = 2                # raw dense streams per parity (k-th edge of row)
GK = 6                  # group-reduction fan-in
MAXTOK = 7936           # per-instruction descriptor limit (< 8192)

_CACHE = {}


def _wrap_idx(tok):
    """Token list -> wrapped [128, n/16] int16 (16-partition wrap, x8)."""
    n = len(tok)
    assert n % 16 == 0
    w = np.asarray(tok, np.int16).reshape(n // 16, 16).T
    return np.tile(w, (8, 1))


def _pad128(n):
    return int(np.ceil(max(n, 1) / 128) * 128)


def _host_prep(features, src, dst):
    src = np.asarray(src).astype(np.int64)
    dst = np.asarray(dst).astype(np.int64)
    feats = np.asarray(features, dtype=np.float32)

    out_deg = np.bincount(src, minlength=N).astype(np.int64)
    in_deg = np.bincount(dst, minlength=N).astype(np.int64)

    dcore = dst // NPC
    dloc = dst - dcore * NPC
    trow = (dcore * ROWS_PC + (dloc >> 1)).astype(np.int64)
    dpar = (dst & 1).astype(np.int64)
    score = src // NPC
    sloc = src - score * NPC

    # rank of each edge within its (src, dst-parity) bucket
    key = src * 2 + dpar
    order = np.argsort(key, kind="stable")
    ks = key[order]
    runstart = np.r_[0, np.flatnonzero(np.diff(ks)) + 1]
    runid = np.cumsum(np.bincount(runstart, minlength=E))[:E] - 1
    nrank = np.empty(E, np.int64)
    nrank[order] = np.arange(E) - runstart[runid]

    per_core = []
    for c in range(NCORES):
        sel = score == c
        # relabel local src nodes by descending degree so high-rank
        # positional streams can be statically trimmed
        nlo = c * NPC
        degtot = np.zeros(NLOC, np.int64)
        degtot[:NPC] = out_deg[nlo:nlo + NPC]
        perm = np.argsort(-degtot, kind="stable")   # new pos -> old local
        rperm = np.empty(NLOC, np.int64)
        rperm[perm] = np.arange(NLOC)               # old local -> new pos
        e_sl = rperm[sloc[sel]]
        e_row = trow[sel]
        e_par = dpar[sel]
        e_rank = nrank[sel]

        # positional: rank < WPOS.  In-instruction duplicate rows are
        # repaired by moving the edge to another rank that is free at its
        # node (iterative, a few rounds); unresolved edges fall to dense.
        pos_mask = e_rank < WPOS
        a_rank = e_rank.copy()
        rng = np.random.default_rng(1234 + c)
        evict = np.zeros(len(e_sl), bool)
        for rnd in range(6):
            # rank usage bitmap per (node, parity)
            used = np.zeros((2, NLOC, WPOS), bool)
            keep = pos_mask & ~evict
            used[e_par[keep], e_sl[keep], a_rank[keep]] = True
            changed = 0
            for par in range(2):
                for j in range(WPOS):
                    m = keep & (e_par == par) & (a_rank == j)
                    if not m.any():
                        continue
                    idxs = np.flatnonzero(m)
                    rows = e_row[idxs]
                    o = np.argsort(rows, kind="stable")
                    dupmask = np.zeros(len(idxs), bool)
                    dupmask[o[1:]] = rows[o[1:]] == rows[o[:-1]]
                    losers = idxs[dupmask]
                    if not len(losers):
                        continue
                    if rnd == 5:
                        evict[losers] = True
                        continue
                    # move each loser to a random free rank at its node
                    free = ~used[par, e_sl[losers], :]      # [L, WPOS]
                    anyfree = free.any(axis=1)
                    pick = np.argmax(
                        free * rng.random((len(losers), WPOS)), axis=1)
                    mv = anyfree
                    lm = losers[mv]
                    used[par, e_sl[lm], a_rank[lm]] = False
                    a_rank[lm] = pick[mv]
                    used[par, e_sl[lm], a_rank[lm]] = True
                    evict[losers[~anyfree]] = True
                    changed += int(mv.sum())
            if changed == 0:
                break
        e_rank = a_rank
        use_pos = pos_mask & ~evict

        junk_row = c * ROWS_PC + PAIRS_L
        pos_idx = np.full((2, WPOS, NLOC), junk_row, np.int64)
        pos_edge = np.full((2, WPOS, NLOC), -1, np.int64)
        pi = np.flatnonzero(use_pos)
        pos_idx[e_par[pi], e_rank[pi], e_sl[pi]] = e_row[pi]
        pos_edge[e_par[pi], e_rank[pi], e_sl[pi]] = pi

        # cross-rank duplicates inside a 7936-token window -> evict to dense
        NW = WPOS * NLOC
        for par in range(2):
            stream = pos_idx[par].reshape(NW)
            eidx = pos_edge[par].reshape(NW)
            for w0 in range(0, NW, MAXTOK):
                win = stream[w0:w0 + MAXTOK]
                real = np.flatnonzero(win != junk_row)
                rows = win[real]
                o = np.argsort(rows, kind="stable")
                dup = np.zeros(len(real), bool)
                dup[o[1:]] = rows[o[1:]] == rows[o[:-1]]
                losers = real[dup]
                if len(losers):
                    evict[eidx[w0 + losers]] = True
                    win[losers] = junk_row
        use_pos = pos_mask & ~evict

        # dense pool: leftover edges, bucketed per (row, parity)
        di = np.flatnonzero(~use_pos)
        d_row = e_row[di]
        d_par = e_par[di]
        d_src = perm[e_sl[di]] + nlo        # original global src ids
        pc = dict(pos_idx=pos_idx, junk=junk_row)
        for par in range(2):
            m = d_par == par
            rows, srcs = d_row[m], d_src[m]
            o = np.argsort(rows, kind="stable")
            rows, srcs = rows[o], srcs[o]
            rs = np.r_[0, np.flatnonzero(np.diff(rows)) + 1]
            rid = np.cumsum(np.bincount(rs, minlength=len(rows)))[:len(rows)] - 1
            kk = np.arange(len(rows)) - rs[rid]        # rank within row
            cnt = np.diff(np.r_[rs, len(rows)])[rid]   # bucket size per edge
            raw_m = cnt <= NRAW
            raws = []
            for k in range(NRAW):
                s = np.flatnonzero(raw_m & (kk == k))
                raws.append(dict(row=rows[s], src=srcs[s]))
            gs = np.flatnonzero(~raw_m)
            grows, gsrcs, gkk = rows[gs], srcs[gs], kk[gs]
            chunk = gkk // GK
            nchunk = int(chunk.max()) + 1 if len(chunk) else 0
            grp = []
            for ch in range(nchunk):
                s = np.flatnonzero(chunk == ch)
                r = grows[s]
                u, inv = np.unique(r, return_inverse=True)
                mem = gkk[s] - ch * GK
                grp.append(dict(rows=u, ginv=inv, mem=mem, src=gsrcs[s]))
            pc[f"raw{par}"] = raws
            pc[f"grp{par}"] = grp
        pc["perm"] = perm
        per_core.append(pc)

    # uniform sizes across cores
    NW = WPOS * NLOC
    pos_win = []
    for w0 in range(0, NW, MAXTOK):
        pos_win.append(min(MAXTOK, NW - w0))
    raw_sz = {0: [], 1: []}
    grp_sz = {0: [], 1: []}
    for par in range(2):
        nraw = max(len(per_core[c][f"raw{par}"]) for c in range(NCORES))
        for c in range(NCORES):
            while len(per_core[c][f"raw{par}"]) < nraw:
                per_core[c][f"raw{par}"].append(
                    dict(row=np.zeros(0, np.int64), src=np.zeros(0, np.int64)))
        for k in range(nraw):
            mx = max(len(per_core[c][f"raw{par}"][k]["row"])
                     for c in range(NCORES))
            raw_sz[par].append(_pad128(mx))
        ng = max(len(per_core[c][f"grp{par}"]) for c in range(NCORES))
        for ch in range(ng):
            mx = max((len(per_core[c][f"grp{par}"][ch]["rows"])
                      if ch < len(per_core[c][f"grp{par}"]) else 0)
                     for c in range(NCORES))
            grp_sz[par].append(_pad128(mx))
    # split any oversized raw stream at the instruction cap; pieces of a
    # duplicate-free stream stay duplicate-free
    for par in range(2):
        while True:
            for k, sz in enumerate(raw_sz[par]):
                if sz > MAXTOK:
                    for c in range(NCORES):
                        d = per_core[c][f"raw{par}"][k]
                        cut = min(MAXTOK, len(d["row"]))
                        per_core[c][f"raw{par}"].insert(
                            k + 1,
                            dict(row=d["row"][cut:], src=d["src"][cut:]))
                        d["row"], d["src"] = d["row"][:cut], d["src"][:cut]
                    raw_sz[par][k] = MAXTOK
                    raw_sz[par].insert(k + 1, _pad128(max(
                        len(per_core[c][f"raw{par}"][k + 1]["row"])
                        for c in range(NCORES))))
                    break
            else:
                break

    CDR = sum(raw_sz[0]) + sum(raw_sz[1])        # raw dense tokens
    NGT = sum(grp_sz[0]) + sum(grp_sz[1])        # group tokens (reduced)

    in_maps = []
    for c in range(NCORES):
        pc = per_core[c]
        junk = pc["junk"]

        idx_cols = []
        for par in range(2):
            stream = pc["pos_idx"][par].reshape(WPOS * NLOC)
            w0 = 0
            for n in pos_win:
                idx_cols.append(_wrap_idx(stream[w0:w0 + n]))
                w0 += n

        dmsg = np.zeros((CDR, D), np.float32)
        ddeg = np.ones(CDR, np.float64)
        coff = 0
        for par in range(2):
            for k in range(len(raw_sz[par])):
                d = pc[f"raw{par}"][k]
                n = raw_sz[par][k]
                tok = np.full(n, junk, np.int64)
                tok[: len(d["row"])] = d["row"]
                idx_cols.append(_wrap_idx(tok))
                if len(d["src"]):
                    dmsg[coff:coff + len(d["src"])] = feats[d["src"]]
                    ddeg[coff:coff + len(d["src"])] = out_deg[d["src"]]
                coff += n

        # group region: [NGT, D, GK], member index innermost
        gmsg = np.zeros((NGT, D, GK), np.float32)
        gdeg = np.ones((NGT, GK), np.float64)
        goff = 0
        for par in range(2):
            for ch in range(len(grp_sz[par])):
                n = grp_sz[par][ch]
                tok = np.full(n, junk, np.int64)
                if ch < len(pc[f"grp{par}"]):
                    g = pc[f"grp{par}"][ch]
                    tok[: len(g["rows"])] = g["rows"]
                    gi = goff + g["ginv"]
                    gmsg[gi, :, g["mem"]] = feats[g["src"]]
                    gdeg[gi, g["mem"]] = out_deg[g["src"]]
                idx_cols.append(_wrap_idx(tok))
                goff += n
        sidx = np.concatenate(idx_cols, axis=1)

        dmsg_dev = np.ascontiguousarray(
            dmsg.reshape(CDR // 128, 128, D).transpose(1, 0, 2)
        ).astype(np.float16)
        ddeg_dev = np.ascontiguousarray(
            ddeg.reshape(CDR // 128, 128).T).astype(np.float16)
        gmsg_dev = np.ascontiguousarray(
            gmsg.reshape(NGT // 128, 128, D, GK).transpose(1, 0, 2, 3)
        ).astype(np.float16)
        gdeg_dev = np.ascontiguousarray(
            gdeg.reshape(NGT // 128, 128, GK).transpose(1, 0, 2)
        ).astype(np.float16)

        nlo = c * NPC
        perm = pc["perm"]
        fc = np.zeros((NLOC, D), np.float16)
        fc[:NPC] = feats[nlo:nlo + NPC]
        fc = fc[perm]                       # degree-sorted order
        featc = np.ascontiguousarray(
            fc.reshape(BLOCKS, 128, D).transpose(1, 0, 2))
        odg = np.zeros(NLOC, np.float16)
        odg[:NPC] = out_deg[nlo:nlo + NPC]
        odg = odg[perm]
        outdegc = np.ascontiguousarray(odg.reshape(BLOCKS, 128).T)

        fpl = np.zeros((ROWS_PC, 2 * D), np.float16)
        fpl[:PAIRS_L, 0:D] = feats[nlo:nlo + NPC:2]
        fpl[:PAIRS_L, D:2 * D] = feats[nlo + 1:nlo + NPC:2]
        featp = np.ascontiguousarray(
            fpl.reshape(ROWS_PC // 128, 128, 2 * D).transpose(1, 0, 2))
        idg = np.zeros((ROWS_PC, 2), np.float16)
        idg[:PAIRS_L, 0] = in_deg[nlo:nlo + NPC:2]
        idg[:PAIRS_L, 1] = in_deg[nlo + 1:nlo + NPC:2]
        indegp = np.ascontiguousarray(
            idg.reshape(ROWS_PC // 128, 128, 2).transpose(1, 0, 2))

        in_maps.append(dict(sidx=sidx, dmsg=dmsg_dev, ddeg=ddeg_dev,
                            gmsg=gmsg_dev, gdeg=gdeg_dev,
                            featc=featc, outdegc=outdegc,
                            featp=featp, indegp=indegp))

    meta = dict(raw_sz0=tuple(raw_sz[0]), raw_sz1=tuple(raw_sz[1]),
                grp_sz0=tuple(grp_sz[0]), grp_sz1=tuple(grp_sz[1]),
                pos_win=tuple(pos_win),
                CDR=CDR, NGT=NGT)
    return in_maps, meta


# ---------------------------------------------------------------------------

def _build_program(meta):
    import os

    import concourse.tile as tile
    from concourse import bacc, mybir

    f16 = mybir.dt.float16
    f32 = mybir.dt.float32
    i16 = mybir.dt.int16
    AF = mybir.ActivationFunctionType
    OP = mybir.AluOpType
    AX = mybir.AxisListType

    raw_sz = {0: meta["raw_sz0"], 1: meta["raw_sz1"]}
    grp_sz = {0: meta["grp_sz0"], 1: meta["grp_sz1"]}
    pos_win = meta["pos_win"]
    CDR, NGT = meta["CDR"], meta["NGT"]
    TOTIDX = 2 * WPOS * NLOC + CDR + NGT
    PB = ROWS_PC // 128

    nc = bacc.Bacc("TRN2", target_bir_lowering=False, debug=False,
                   num_devices=NCORES, num_swdge_queues=2)

    sidxD = nc.dram_tensor("sidx", [128, TOTIDX // 16], i16,
                           kind="ExternalInput").ap()
    dmsgD = nc.dram_tensor("dmsg", [128, CDR // 128, D], f16,
                           kind="ExternalInput").ap()
    ddegD = nc.dram_tensor("ddeg", [128, CDR // 128], f16,
                           kind="ExternalInput").ap()
    gmsgD = nc.dram_tensor("gmsg", [128, NGT // 128, D, GK], f16,
                           kind="ExternalInput").ap()
    gdegD = nc.dram_tensor("gdeg", [128, NGT // 128, GK], f16,
                           kind="ExternalInput").ap()
    featcD = nc.dram_tensor("featc", [128, BLOCKS, D], f16,
                            kind="ExternalInput").ap()
    outdegcD = nc.dram_tensor("outdegc", [128, BLOCKS], f16,
                              kind="ExternalInput").ap()
    featpD = nc.dram_tensor("featp", [128, ROWS_PC // 128, 2 * D], f16,
                            kind="ExternalInput").ap()
    indegpD = nc.dram_tensor("indegp", [128, ROWS_PC // 128, 2], f16,
                             kind="ExternalInput").ap()
    wb2D = nc.dram_tensor("wb2", [2 * D + 1, 2 * D], f16,
                          kind="ExternalInput").ap()
    identD = nc.dram_tensor("ident", [128, 128], f32,
                            kind="ExternalInput").ap()
    outD = nc.dram_tensor("out", [2 * D, ROWS_PC], f32,
                          kind="ExternalOutput").ap()

    with tile.TileContext(nc) as tc:
        with tc.tile_pool(name="const", bufs=1) as cpool, \
             tc.tile_pool(name="big", bufs=1) as bigpool, \
             tc.tile_pool(name="dram", bufs=1, space="DRAM") as drampool:

            wb2_s = cpool.tile([2 * D + 1, 2 * D], f16, tag="wb2")
            nc.sync.dma_start(out=wb2_s[:], in_=wb2D)
            ident = cpool.tile([128, 128], f32, tag="ident")
            nc.sync.dma_start(out=ident[:], in_=identD)

            # partial table + contiguous zero-init
            ptab = drampool.tile([TROWS, TCOLS], f16)
            ZB = 50
            zer = cpool.tile([128, ZB, TCOLS], f16, tag="zer")
            nc.vector.memset(zer[:], 0.0)
            ptz = ptab.rearrange("(p a) d -> p a d", p=128)
            for z in range(TROWS // 128 // ZB):
                nc.sync.dma_start(out=ptz[:, z * ZB:(z + 1) * ZB, :],
                                  in_=zer[:])

            sidx = bigpool.tile([128, TOTIDX // 16], i16, tag="sidx")
            PC0 = WPOS * NLOC // 16              # O-parity idx col offset
            early = 4 * NLOC // 16
            cuts = [(0, early), (PC0, PC0 + early),
                    (early, PC0), (PC0 + early, TOTIDX // 16)]
            for lo, hi in cuts:
                nc.sync.dma_start(out=sidx[:, lo:hi], in_=sidxD[:, lo:hi])

            # X' = featc * rsqrt(max(outdeg,1))
            featc_s = cpool.tile([128, BLOCKS, D], f16, tag="featc")
            nc.sync.dma_start(out=featc_s[:], in_=featcD)
            odeg = cpool.tile([128, BLOCKS], f16, tag="odeg")
            nc.sync.dma_start(out=odeg[:], in_=outdegcD)
            ci = cpool.tile([128, BLOCKS], f32, tag="ci")
            nc.vector.tensor_copy(ci[:], odeg[:])
            nc.vector.tensor_scalar_max(ci[:], ci[:], 1.0)
            nc.scalar.activation(ci[:], ci[:], AF.Sqrt)
            nc.vector.reciprocal(ci[:], ci[:])
            WCOL = (MAXTOK + 127) // 128          # 62 wrap columns
            xp = bigpool.tile([128, BLOCKS + WCOL, D], f16, tag="xp")
            nc.vector.tensor_tensor(
                xp[:, 0:BLOCKS, :], featc_s[:],
                ci[:, :].unsqueeze(2).to_broadcast([128, BLOCKS, D]),
                OP.mult)
            nc.vector.tensor_copy(xp[:, BLOCKS:BLOCKS + WCOL, :],
                                  xp[:, 0:WCOL, :])

            # ---- scatter chains (one per dst parity) ---------------------
            voff = {0: 0, 1: 64}
            state = dict(icol=0, qn=0)

            def scat(n_tok, in_slice, par):
                c0 = voff[par]
                nc.gpsimd.dma_scatter_add(
                    out_ap=ptab[:, c0:c0 + D],
                    in_ap=in_slice,
                    idxs_ap=sidx[:, state["icol"]:state["icol"] + n_tok // 16],
                    num_idxs=n_tok,
                    num_idxs_reg=n_tok,
                    elem_size=D,
                    elem_step=TCOLS,
                    queue_num=state["qn"] % 2,
                    single_packet=False,
                )
                state["icol"] += n_tok // 16
                state["qn"] += 1

            for par in range(2):
                goff = 0
                for n in pos_win:
                    off = goff % NLOC
                    assert off % 128 == 0
                    oc0 = off // 128
                    scat(n, xp[:, oc0:oc0 + (n + 127) // 128, :], par)
                    goff += n
            # dense raw messages scaled in place
            dmsg_s = bigpool.tile([128, CDR // 128, D], f16, tag="dmsg")
            nc.sync.dma_start(out=dmsg_s[:], in_=dmsgD)
            ddeg_s = cpool.tile([128, CDR // 128], f16, tag="ddeg")
            nc.sync.dma_start(out=ddeg_s[:], in_=ddegD)
            cid = cpool.tile([128, CDR // 128], f32, tag="cid")
            nc.vector.tensor_copy(cid[:], ddeg_s[:])
            nc.vector.tensor_scalar_max(cid[:], cid[:], 1.0)
            nc.scalar.activation(cid[:], cid[:], AF.Sqrt)
            nc.vector.reciprocal(cid[:], cid[:])
            nc.vector.tensor_tensor(
                dmsg_s[:], dmsg_s[:],
                cid[:, :].unsqueeze(2).to_broadcast([128, CDR // 128, D]),
                OP.mult)

            # group region: scale members then GK-way reduce
            GC = NGT // 128
            gmsg_s = bigpool.tile([128, GC, D, GK], f16, tag="gmsg")
            nc.sync.dma_start(out=gmsg_s[:], in_=gmsgD)
            gdeg_s = cpool.tile([128, GC, GK], f16, tag="gdeg")
            nc.sync.dma_start(out=gdeg_s[:], in_=gdegD)
            cig = cpool.tile([128, GC, GK], f32, tag="cig")
            nc.vector.tensor_copy(cig[:], gdeg_s[:])
            nc.vector.tensor_scalar_max(cig[:], cig[:], 1.0)
            nc.scalar.activation(cig[:], cig[:], AF.Sqrt)
            nc.vector.reciprocal(cig[:], cig[:])
            nc.vector.tensor_tensor(
                gmsg_s[:], gmsg_s[:],
                cig[:].unsqueeze(2).to_broadcast([128, GC, D, GK]),
                OP.mult)
            gred32 = cpool.tile([128, GC, D], f32, tag="gred32")
            nc.vector.tensor_reduce(gred32[:], gmsg_s[:], AX.X, OP.add)
            gred = cpool.tile([128, GC, D], f16, tag="gred")
            nc.vector.tensor_copy(gred[:], gred32[:])

            dcol = 0
            for par in range(2):
                for k in range(len(raw_sz[par])):
                    n = raw_sz[par][k]
                    scat(n, dmsg_s[:, dcol:dcol + n // 128, :], par)
                    dcol += n // 128
            gcol = 0
            for par in range(2):
                for ch in range(len(grp_sz[par])):
                    n = grp_sz[par][ch]
                    scat(n, gred[:, gcol:gcol + n // 128, :], par)
                    gcol += n // 128

            # ---- ReduceScatter -------------------------------------------
            rsout = drampool.tile([ROWS_PC, TCOLS], f16)
            if os.environ.get("V2_SKIP_RS"):
                nc.gpsimd.dma_start(out=rsout[:], in_=ptab[0:ROWS_PC, :])
            else:
                nc.gpsimd.collective_compute(
                    "ReduceScatter",
                    mybir.AluOpType.add,
                    replica_groups=[list(range(NCORES))],
                    ins=[ptab.opt()],
                    outs=[rsout.opt()],
                )

            # ---- tail ----------------------------------------------------
            rs_s = bigpool.tile([128, PB, TCOLS], f16, tag="rs")
            nc.sync.dma_start(
                out=rs_s[:], in_=rsout.rearrange("(a p) d -> p a d", p=128))
            featp_s = cpool.tile([128, PB, 2 * D], f16, tag="featp")
            nc.sync.dma_start(out=featp_s[:], in_=featpD)
            indegp_s = cpool.tile([128, PB, 2], f16, tag="indegp")
            nc.sync.dma_start(out=indegp_s[:], in_=indegpD)

            idf = cpool.tile([128, PB, 2], f32, tag="idf")
            nc.vector.tensor_copy(idf[:], indegp_s[:])
            maskp = cpool.tile([128, PB, 2], f32, tag="maskp")
            nc.vector.tensor_scalar(maskp[:], idf[:], 0.0, None, OP.is_gt)
            cjp = cpool.tile([128, PB, 2], f32, tag="cjp")
            nc.vector.tensor_scalar_max(cjp[:], idf[:], 1.0)
            nc.scalar.activation(cjp[:], cjp[:], AF.Sqrt)
            nc.vector.reciprocal(cjp[:], cjp[:])
            nc.vector.tensor_mul(cjp[:], cjp[:], maskp[:])
            im1 = cpool.tile([128, PB, 2], f32, tag="im1")
            nc.vector.tensor_scalar(im1[:], maskp[:], -1.0, 1.0,
                                    OP.mult, OP.add)

            # h = agg * cj + featp * (1 - mask), in 5-block chunks so the
            # transposes start while later chunks still blend
            hp = bigpool.tile([128, PB, 2 * D], f32, tag="hp")
            tmp = bigpool.tile([128, PB, 2 * D], f32, tag="tmp")
            hT2 = bigpool.tile([2 * D + 1, ROWS_PC], f16, tag="hT2")
            nc.vector.memset(hT2[:, :], 1.0)
            CB = 5
            with tc.tile_pool(name="tp", bufs=4, space="PSUM") as tppool, \
                 tc.tile_pool(name="oc", bufs=3) as ocpool:
                for g0 in range(0, PB, CB):
                    bs = slice(g0, g0 + CB)
                    for par, c0 in ((0, 0), (1, 64)):
                        sl = slice(par * D, par * D + D)
                        nc.vector.tensor_tensor(
                            hp[:, bs, sl], rs_s[:, bs, c0:c0 + D],
                            cjp[:, bs, par:par + 1].to_broadcast(
                                [128, CB, D]), OP.mult)
                        nc.vector.tensor_tensor(
                            tmp[:, bs, sl], featp_s[:, bs, sl],
                            im1[:, bs, par:par + 1].to_broadcast(
                                [128, CB, D]), OP.mult)
                    nc.vector.tensor_add(hp[:, bs, :], hp[:, bs, :],
                                         tmp[:, bs, :])
                    for b in range(g0, g0 + CB):
                        tp = tppool.tile([2 * D, 128], f32, tag="tp")
                        nc.tensor.transpose(tp[:], hp[:, b, :], ident[:])
                        nc.vector.tensor_copy(
                            hT2[0:2 * D, b * 128:(b + 1) * 128], tp[:])

                CH = 512
                nch = (ROWS_PC + CH - 1) // CH
                for i in range(nch):
                    lo = i * CH
                    hi = min(lo + CH, ROWS_PC)
                    po = tppool.tile([2 * D, CH], f32, tag="po")
                    nc.tensor.matmul(po[:, 0:hi - lo], lhsT=wb2_s[:],
                                     rhs=hT2[:, lo:hi], start=True, stop=True)
                    oc = ocpool.tile([2 * D, CH], f32, tag="oc")
                    nc.scalar.activation(oc[:, 0:hi - lo], po[:, 0:hi - lo],
                                         AF.Relu)
                    nc.sync.dma_start(out=outD[:, lo:hi], in_=oc[:, 0:hi - lo])

    nc.compile()
    return nc


# ---------------------------------------------------------------------------

def kernel(features, src, dst, W, b):
    from concourse.bass_utils import run_bass_kernel_spmd

    in_maps, meta = _host_prep(features, src, dst)

    key = tuple(sorted(meta.items()))
    if key not in _CACHE:
        _CACHE[key] = _build_program(meta)
    nc = _CACHE[key]

    Wt = np.asarray(W, np.float32).T          # [48 in, 48 out]
    bv = np.asarray(b, np.float32)
    wb2 = np.zeros((2 * D + 1, 2 * D), np.float16)
    wb2[0:D, 0:D] = Wt
    wb2[D:2 * D, D:2 * D] = Wt
    wb2[2 * D, 0:D] = bv
    wb2[2 * D, D:2 * D] = bv
    ident = np.eye(128, dtype=np.float32)
    for m in in_maps:
        m["wb2"] = wb2
        m["ident"] = ident

    res = run_bass_kernel_spmd(nc, in_maps, core_ids=list(range(NCORES)))
    globals()["LAST_RESULTS"] = res

    out = np.empty((N, D), np.float32)
    for c in range(NCORES):
        o = res.results[c]["out"].astype(np.float32)
        nlo = c * NPC
        out[nlo:nlo + NPC:2] = o[0:D, :PAIRS_L].T
        out[nlo + 1:nlo + NPC:2] = o[D:2 * D, :PAIRS_L].T
    return np.ascontiguousarray(out, dtype=np.float32)


# revision 6
# speedup vs baseline: 1.0090x; 1.0090x over previous
"""GCN layer on 8 TRN2 cores — scatter-add architecture.

Sharding: cores own contiguous SRC ranges (edge/data parallel per the
sharding hint). Each core computes scaled messages X'[src]=feat*ci for its
~200k out-edges and dma_scatter_add's them into a per-core partial
aggregate table in DRAM (fp16 pair-rows of 256 B; even dst nodes at cols
0:48, odd at 64:112). A ReduceScatter (add) over the 8 partial tables
gives each core the summed aggregates for its 6250-dst slice; the tail
(cj scaling, zero-in-degree fallback, 48x48 linear + relu) runs locally.

Message expansion is free for the first WPOS edges per (src, dst-parity):
scatter instruction r reads the X' SBUF tile directly as its token payload
(token position == local node id).  Remaining edges are host-pre-gathered
raw feature rows, scaled by rsqrt(out_deg) on device; rows receiving many
such edges are pre-summed on the vector engine (8-way groups) so each
leftover stream stays large.

dma_scatter_add loses duplicate target rows *within* one instruction
(parallel read-modify-write against a snapshot), so every instruction's
rows are made unique by construction: positional streams evict in-
instruction duplicates to the dense path; dense streams take the k-th
edge of each (row, parity) bucket; group-reduction emits one token per
row.  Instructions on the same parity view are serialized by the
framework's WAW dependency, which makes cross-instruction accumulation
exact; the two parity views are byte-disjoint so their chains overlap.
"""

import numpy as np

N = 50000
E = 1600000
D = 48
NCORES = 8
NPC = 6250              # real nodes per core
NLOC = 6272             # padded local nodes (49 * 128)
BLOCKS = 49
PAIRS_L = 3125          # real local pairs
ROWS_PC = 3200          # table rows per core (incl. junk rows)
TROWS = ROWS_PC * NCORES
TCOLS = 128             # 256 B rows: E@0:48  O@64:112
WPOS = 14               # positional ranks per (node, dst-parity)
NRAW = 2                # raw dense streams per parity (k-th edge of row)
GK = 6                  # group-reduction fan-in
MAXTOK = 7936           # per-instruction descriptor limit (< 8192)

_CACHE = {}


def _wrap_idx(tok):
    """Token list -> wrapped [128, n/16] int16 (16-partition wrap, x8)."""
    n = len(tok)
    assert n % 16 == 0
    w = np.asarray(tok, np.int16).reshape(n // 16, 16).T
    return np.tile(w, (8, 1))


def _pad128(n):
    return int(np.ceil(max(n, 1) / 128) * 128)


def _host_prep(features, src, dst):
    src = np.asarray(src).astype(np.int64)
    dst = np.asarray(dst).astype(np.int64)
    feats = np.asarray(features, dtype=np.float32)

    out_deg = np.bincount(src, minlength=N).astype(np.int64)
    in_deg = np.bincount(dst, minlength=N).astype(np.int64)

    dcore = dst // NPC
    dloc = dst - dcore * NPC
    trow = (dcore * ROWS_PC + (dloc >> 1)).astype(np.int64)
    dpar = (dst & 1).astype(np.int64)
    score = src // NPC
    sloc = src - score * NPC

    # rank of each edge within its (src, dst-parity) bucket
    key = src * 2 + dpar
    order = np.argsort(key, kind="stable")
    ks = key[order]
    runstart = np.r_[0, np.flatnonzero(np.diff(ks)) + 1]
    runid = np.cumsum(np.bincount(runstart, minlength=E))[:E] - 1
    nrank = np.empty(E, np.int64)
    nrank[order] = np.arange(E) - runstart[runid]

    per_core = []
    for c in range(NCORES):
        sel = score == c
        # relabel local src nodes by descending degree so high-rank
        # positional streams can be statically trimmed
        nlo = c * NPC
        degtot = np.zeros(NLOC, np.int64)
        degtot[:NPC] = out_deg[nlo:nlo + NPC]
        perm = np.argsort(-degtot, kind="stable")   # new pos -> old local
        rperm = np.empty(NLOC, np.int64)
        rperm[perm] = np.arange(NLOC)               # old local -> new pos
        e_sl = rperm[sloc[sel]]
        e_row = trow[sel]
        e_par = dpar[sel]
        e_rank = nrank[sel]

        # positional: rank < WPOS.  In-instruction duplicate rows are
        # repaired by moving the edge to another rank that is free at its
        # node (iterative, a few rounds); unresolved edges fall to dense.
        pos_mask = e_rank < WPOS
        a_rank = e_rank.copy()
        rng = np.random.default_rng(1234 + c)
        evict = np.zeros(len(e_sl), bool)
        for rnd in range(6):
            # rank usage bitmap per (node, parity)
            used = np.zeros((2, NLOC, WPOS), bool)
            keep = pos_mask & ~evict
            used[e_par[keep], e_sl[keep], a_rank[keep]] = True
            changed = 0
            for par in range(2):
                for j in range(WPOS):
                    m = keep & (e_par == par) & (a_rank == j)
                    if not m.any():
                        continue
                    idxs = np.flatnonzero(m)
                    rows = e_row[idxs]
                    o = np.argsort(rows, kind="stable")
                    dupmask = np.zeros(len(idxs), bool)
                    dupmask[o[1:]] = rows[o[1:]] == rows[o[:-1]]
                    losers = idxs[dupmask]
                    if not len(losers):
                        continue
                    if rnd == 5:
                        evict[losers] = True
                        continue
                    # move each loser to a random free rank at its node
                    free = ~used[par, e_sl[losers], :]      # [L, WPOS]
                    anyfree = free.any(axis=1)
                    pick = np.argmax(
                        free * rng.random((len(losers), WPOS)), axis=1)
                    mv = anyfree
                    lm = losers[mv]
                    used[par, e_sl[lm], a_rank[lm]] = False
                    a_rank[lm] = pick[mv]
                    used[par, e_sl[lm], a_rank[lm]] = True
                    evict[losers[~anyfree]] = True
                    changed += int(mv.sum())
            if changed == 0:
                break
        e_rank = a_rank
        use_pos = pos_mask & ~evict

        junk_row = c * ROWS_PC + PAIRS_L
        pos_idx = np.full((2, WPOS, NLOC), junk_row, np.int64)
        pi = np.flatnonzero(use_pos)
        pos_idx[e_par[pi], e_rank[pi], e_sl[pi]] = e_row[pi]
        pos_last = np.zeros((2, WPOS), np.int64)
        for par in range(2):
            for j in range(WPOS):
                m = (e_par[pi] == par) & (e_rank[pi] == j)
                pos_last[par, j] = (e_sl[pi][m].max() + 1) if m.any() else 128

        # dense pool: leftover edges, bucketed per (row, parity)
        di = np.flatnonzero(~use_pos)
        d_row = e_row[di]
        d_par = e_par[di]
        d_src = perm[e_sl[di]] + nlo        # original global src ids
        pc = dict(pos_idx=pos_idx, junk=junk_row)
        for par in range(2):
            m = d_par == par
            rows, srcs = d_row[m], d_src[m]
            o = np.argsort(rows, kind="stable")
            rows, srcs = rows[o], srcs[o]
            rs = np.r_[0, np.flatnonzero(np.diff(rows)) + 1]
            rid = np.cumsum(np.bincount(rs, minlength=len(rows)))[:len(rows)] - 1
            kk = np.arange(len(rows)) - rs[rid]        # rank within row
            cnt = np.diff(np.r_[rs, len(rows)])[rid]   # bucket size per edge
            raw_m = cnt <= NRAW
            raws = []
            for k in range(NRAW):
                s = np.flatnonzero(raw_m & (kk == k))
                raws.append(dict(row=rows[s], src=srcs[s]))
            gs = np.flatnonzero(~raw_m)
            grows, gsrcs, gkk = rows[gs], srcs[gs], kk[gs]
            chunk = gkk // GK
            nchunk = int(chunk.max()) + 1 if len(chunk) else 0
            grp = []
            for ch in range(nchunk):
                s = np.flatnonzero(chunk == ch)
                r = grows[s]
                u, inv = np.unique(r, return_inverse=True)
                mem = gkk[s] - ch * GK
                grp.append(dict(rows=u, ginv=inv, mem=mem, src=gsrcs[s]))
            pc[f"raw{par}"] = raws
            pc[f"grp{par}"] = grp
        pc["pos_last"] = pos_last
        pc["perm"] = perm
        per_core.append(pc)

    # uniform sizes across cores
    pos_len = np.zeros((2, WPOS), np.int64)
    for par in range(2):
        for j in range(WPOS):
            pos_len[par, j] = _pad128(max(
                per_core[c]["pos_last"][par, j] for c in range(NCORES)))
    raw_sz = {0: [], 1: []}
    grp_sz = {0: [], 1: []}
    for par in range(2):
        nraw = max(len(per_core[c][f"raw{par}"]) for c in range(NCORES))
        for c in range(NCORES):
            while len(per_core[c][f"raw{par}"]) < nraw:
                per_core[c][f"raw{par}"].append(
                    dict(row=np.zeros(0, np.int64), src=np.zeros(0, np.int64)))
        for k in range(nraw):
            mx = max(len(per_core[c][f"raw{par}"][k]["row"])
                     for c in range(NCORES))
            raw_sz[par].append(_pad128(mx))
        ng = max(len(per_core[c][f"grp{par}"]) for c in range(NCORES))
        for ch in range(ng):
            mx = max((len(per_core[c][f"grp{par}"][ch]["rows"])
                      if ch < len(per_core[c][f"grp{par}"]) else 0)
                     for c in range(NCORES))
            grp_sz[par].append(_pad128(mx))
    # split any oversized raw stream at the instruction cap; pieces of a
    # duplicate-free stream stay duplicate-free
    for par in range(2):
        while True:
            for k, sz in enumerate(raw_sz[par]):
                if sz > MAXTOK:
                    for c in range(NCORES):
                        d = per_core[c][f"raw{par}"][k]
                        cut = min(MAXTOK, len(d["row"]))
                        per_core[c][f"raw{par}"].insert(
                            k + 1,
                            dict(row=d["row"][cut:], src=d["src"][cut:]))
                        d["row"], d["src"] = d["row"][:cut], d["src"][:cut]
                    raw_sz[par][k] = MAXTOK
                    raw_sz[par].insert(k + 1, _pad128(max(
                        len(per_core[c][f"raw{par}"][k + 1]["row"])
                        for c in range(NCORES))))
                    break
            else:
                break

    CDR = sum(raw_sz[0]) + sum(raw_sz[1])        # raw dense tokens
    NGT = sum(grp_sz[0]) + sum(grp_sz[1])        # group tokens (reduced)

    in_maps = []
    for c in range(NCORES):
        pc = per_core[c]
        junk = pc["junk"]

        idx_cols = []
        for par in range(2):
            for j in range(WPOS):
                idx_cols.append(
                    _wrap_idx(pc["pos_idx"][par, j][: pos_len[par, j]]))

        dmsg = np.zeros((CDR, D), np.float32)
        ddeg = np.ones(CDR, np.float64)
        coff = 0
        for par in range(2):
            for k in range(len(raw_sz[par])):
                d = pc[f"raw{par}"][k]
                n = raw_sz[par][k]
                tok = np.full(n, junk, np.int64)
                tok[: len(d["row"])] = d["row"]
                idx_cols.append(_wrap_idx(tok))
                if len(d["src"]):
                    dmsg[coff:coff + len(d["src"])] = feats[d["src"]]
                    ddeg[coff:coff + len(d["src"])] = out_deg[d["src"]]
                coff += n

        # group region: [NGT, D, GK], member index innermost
        gmsg = np.zeros((NGT, D, GK), np.float32)
        gdeg = np.ones((NGT, GK), np.float64)
        goff = 0
        for par in range(2):
            for ch in range(len(grp_sz[par])):
                n = grp_sz[par][ch]
                tok = np.full(n, junk, np.int64)
                if ch < len(pc[f"grp{par}"]):
                    g = pc[f"grp{par}"][ch]
                    tok[: len(g["rows"])] = g["rows"]
                    gi = goff + g["ginv"]
                    gmsg[gi, :, g["mem"]] = feats[g["src"]]
                    gdeg[gi, g["mem"]] = out_deg[g["src"]]
                idx_cols.append(_wrap_idx(tok))
                goff += n
        sidx = np.concatenate(idx_cols, axis=1)

        dmsg_dev = np.ascontiguousarray(
            dmsg.reshape(CDR // 128, 128, D).transpose(1, 0, 2)
        ).astype(np.float16)
        ddeg_dev = np.ascontiguousarray(
            ddeg.reshape(CDR // 128, 128).T).astype(np.float16)
        gmsg_dev = np.ascontiguousarray(
            gmsg.reshape(NGT // 128, 128, D, GK).transpose(1, 0, 2, 3)
        ).astype(np.float16)
        gdeg_dev = np.ascontiguousarray(
            gdeg.reshape(NGT // 128, 128, GK).transpose(1, 0, 2)
        ).astype(np.float16)

        nlo = c * NPC
        perm = pc["perm"]
        fc = np.zeros((NLOC, D), np.float16)
        fc[:NPC] = feats[nlo:nlo + NPC]
        fc = fc[perm]                       # degree-sorted order
        featc = np.ascontiguousarray(
            fc.reshape(BLOCKS, 128, D).transpose(1, 0, 2))
        odg = np.zeros(NLOC, np.float16)
        odg[:NPC] = out_deg[nlo:nlo + NPC]
        odg = odg[perm]
        outdegc = np.ascontiguousarray(odg.reshape(BLOCKS, 128).T)

        fpl = np.zeros((ROWS_PC, 2 * D), np.float16)
        fpl[:PAIRS_L, 0:D] = feats[nlo:nlo + NPC:2]
        fpl[:PAIRS_L, D:2 * D] = feats[nlo + 1:nlo + NPC:2]
        featp = np.ascontiguousarray(
            fpl.reshape(ROWS_PC // 128, 128, 2 * D).transpose(1, 0, 2))
        idg = np.zeros((ROWS_PC, 2), np.float16)
        idg[:PAIRS_L, 0] = in_deg[nlo:nlo + NPC:2]
        idg[:PAIRS_L, 1] = in_deg[nlo + 1:nlo + NPC:2]
        indegp = np.ascontiguousarray(
            idg.reshape(ROWS_PC // 128, 128, 2).transpose(1, 0, 2))

        in_maps.append(dict(sidx=sidx, dmsg=dmsg_dev, ddeg=ddeg_dev,
                            gmsg=gmsg_dev, gdeg=gdeg_dev,
                            featc=featc, outdegc=outdegc,
                            featp=featp, indegp=indegp))

    meta = dict(raw_sz0=tuple(raw_sz[0]), raw_sz1=tuple(raw_sz[1]),
                grp_sz0=tuple(grp_sz[0]), grp_sz1=tuple(grp_sz[1]),
                pos_len=tuple(map(tuple, pos_len.tolist())),
                CDR=CDR, NGT=NGT)
    return in_maps, meta


# ---------------------------------------------------------------------------

def _build_program(meta):
    import os

    import concourse.tile as tile
    from concourse import bacc, mybir

    f16 = mybir.dt.float16
    f32 = mybir.dt.float32
    i16 = mybir.dt.int16
    AF = mybir.ActivationFunctionType
    OP = mybir.AluOpType
    AX = mybir.AxisListType

    raw_sz = {0: meta["raw_sz0"], 1: meta["raw_sz1"]}
    grp_sz = {0: meta["grp_sz0"], 1: meta["grp_sz1"]}
    pos_len = meta["pos_len"]
    CDR, NGT = meta["CDR"], meta["NGT"]
    TOTIDX = sum(pos_len[0]) + sum(pos_len[1]) + CDR + NGT
    PB = ROWS_PC // 128

    nc = bacc.Bacc("TRN2", target_bir_lowering=False, debug=False,
                   num_devices=NCORES, num_swdge_queues=2)

    sidxD = nc.dram_tensor("sidx", [128, TOTIDX // 16], i16,
                           kind="ExternalInput").ap()
    dmsgD = nc.dram_tensor("dmsg", [128, CDR // 128, D], f16,
                           kind="ExternalInput").ap()
    ddegD = nc.dram_tensor("ddeg", [128, CDR // 128], f16,
                           kind="ExternalInput").ap()
    gmsgD = nc.dram_tensor("gmsg", [128, NGT // 128, D, GK], f16,
                           kind="ExternalInput").ap()
    gdegD = nc.dram_tensor("gdeg", [128, NGT // 128, GK], f16,
                           kind="ExternalInput").ap()
    featcD = nc.dram_tensor("featc", [128, BLOCKS, D], f16,
                            kind="ExternalInput").ap()
    outdegcD = nc.dram_tensor("outdegc", [128, BLOCKS], f16,
                              kind="ExternalInput").ap()
    featpD = nc.dram_tensor("featp", [128, ROWS_PC // 128, 2 * D], f16,
                            kind="ExternalInput").ap()
    indegpD = nc.dram_tensor("indegp", [128, ROWS_PC // 128, 2], f16,
                             kind="ExternalInput").ap()
    wb2D = nc.dram_tensor("wb2", [2 * D + 1, 2 * D], f16,
                          kind="ExternalInput").ap()
    identD = nc.dram_tensor("ident", [128, 128], f32,
                            kind="ExternalInput").ap()
    outD = nc.dram_tensor("out", [2 * D, ROWS_PC], f32,
                          kind="ExternalOutput").ap()

    with tile.TileContext(nc) as tc:
        with tc.tile_pool(name="const", bufs=1) as cpool, \
             tc.tile_pool(name="big", bufs=1) as bigpool, \
             tc.tile_pool(name="dram", bufs=1, space="DRAM") as drampool:

            wb2_s = cpool.tile([2 * D + 1, 2 * D], f16, tag="wb2")
            nc.sync.dma_start(out=wb2_s[:], in_=wb2D)
            ident = cpool.tile([128, 128], f32, tag="ident")
            nc.sync.dma_start(out=ident[:], in_=identD)

            # partial table + contiguous zero-init
            ptab = drampool.tile([TROWS, TCOLS], f16)
            ZB = 50
            zer = cpool.tile([128, ZB, TCOLS], f16, tag="zer")
            nc.vector.memset(zer[:], 0.0)
            ptz = ptab.rearrange("(p a) d -> p a d", p=128)
            for z in range(TROWS // 128 // ZB):
                nc.sync.dma_start(out=ptz[:, z * ZB:(z + 1) * ZB, :],
                                  in_=zer[:])

            sidx = bigpool.tile([128, TOTIDX // 16], i16, tag="sidx")
            PC0 = sum(pos_len[0]) // 16          # O-parity idx col offset
            early = 4 * NLOC // 16
            cuts = [(0, early), (PC0, PC0 + early),
                    (early, PC0), (PC0 + early, TOTIDX // 16)]
            for lo, hi in cuts:
                nc.sync.dma_start(out=sidx[:, lo:hi], in_=sidxD[:, lo:hi])

            # X' = featc * rsqrt(max(outdeg,1))
            featc_s = cpool.tile([128, BLOCKS, D], f16, tag="featc")
            nc.sync.dma_start(out=featc_s[:], in_=featcD)
            odeg = cpool.tile([128, BLOCKS], f16, tag="odeg")
            nc.sync.dma_start(out=odeg[:], in_=outdegcD)
            ci = cpool.tile([128, BLOCKS], f32, tag="ci")
            nc.vector.tensor_copy(ci[:], odeg[:])
            nc.vector.tensor_scalar_max(ci[:], ci[:], 1.0)
            nc.scalar.activation(ci[:], ci[:], AF.Sqrt)
            nc.vector.reciprocal(ci[:], ci[:])
            xp = bigpool.tile([128, BLOCKS, D], f16, tag="xp")
            nc.vector.tensor_tensor(
                xp[:], featc_s[:],
                ci[:, :].unsqueeze(2).to_broadcast([128, BLOCKS, D]),
                OP.mult)

            # ---- scatter chains (one per dst parity) ---------------------
            voff = {0: 0, 1: 64}
            state = dict(icol=0, qn=0)

            def scat(n_tok, in_slice, par):
                c0 = voff[par]
                nc.gpsimd.dma_scatter_add(
                    out_ap=ptab[:, c0:c0 + D],
                    in_ap=in_slice,
                    idxs_ap=sidx[:, state["icol"]:state["icol"] + n_tok // 16],
                    num_idxs=n_tok,
                    num_idxs_reg=n_tok,
                    elem_size=D,
                    elem_step=TCOLS,
                    queue_num=state["qn"] % 2,
                    single_packet=False,
                )
                state["icol"] += n_tok // 16
                state["qn"] += 1

            for par in range(2):
                for j in range(WPOS):
                    n = pos_len[par][j]
                    scat(n, xp[:, 0:n // 128, :], par)
            # dense raw messages scaled in place
            dmsg_s = bigpool.tile([128, CDR // 128, D], f16, tag="dmsg")
            nc.sync.dma_start(out=dmsg_s[:], in_=dmsgD)
            ddeg_s = cpool.tile([128, CDR // 128], f16, tag="ddeg")
            nc.sync.dma_start(out=ddeg_s[:], in_=ddegD)
            cid = cpool.tile([128, CDR // 128], f32, tag="cid")
            nc.vector.tensor_copy(cid[:], ddeg_s[:])
            nc.vector.tensor_scalar_max(cid[:], cid[:], 1.0)
            nc.scalar.activation(cid[:], cid[:], AF.Sqrt)
            nc.vector.reciprocal(cid[:], cid[:])
            nc.vector.tensor_tensor(
                dmsg_s[:], dmsg_s[:],
                cid[:, :].unsqueeze(2).to_broadcast([128, CDR // 128, D]),
                OP.mult)

            # group region: scale members then GK-way reduce
            GC = NGT // 128
            gmsg_s = bigpool.tile([128, GC, D, GK], f16, tag="gmsg")
            nc.sync.dma_start(out=gmsg_s[:], in_=gmsgD)
            gdeg_s = cpool.tile([128, GC, GK], f16, tag="gdeg")
            nc.sync.dma_start(out=gdeg_s[:], in_=gdegD)
            cig = cpool.tile([128, GC, GK], f32, tag="cig")
            nc.vector.tensor_copy(cig[:], gdeg_s[:])
            nc.vector.tensor_scalar_max(cig[:], cig[:], 1.0)
            nc.scalar.activation(cig[:], cig[:], AF.Sqrt)
            nc.vector.reciprocal(cig[:], cig[:])
            nc.vector.tensor_tensor(
                gmsg_s[:], gmsg_s[:],
                cig[:].unsqueeze(2).to_broadcast([128, GC, D, GK]),
                OP.mult)
            gred32 = cpool.tile([128, GC, D], f32, tag="gred32")
            nc.vector.tensor_reduce(gred32[:], gmsg_s[:], AX.X, OP.add)
            gred = cpool.tile([128, GC, D], f16, tag="gred")
            nc.vector.tensor_copy(gred[:], gred32[:])

            dcol = 0
            for par in range(2):
                for k in range(len(raw_sz[par])):
                    n = raw_sz[par][k]
                    scat(n, dmsg_s[:, dcol:dcol + n // 128, :], par)
                    dcol += n // 128
            gcol = 0
            for par in range(2):
                for ch in range(len(grp_sz[par])):
                    n = grp_sz[par][ch]
                    scat(n, gred[:, gcol:gcol + n // 128, :], par)
                    gcol += n // 128

            # ---- ReduceScatter -------------------------------------------
            rsout = drampool.tile([ROWS_PC, TCOLS], f16)
            if os.environ.get("V2_SKIP_RS"):
                nc.gpsimd.dma_start(out=rsout[:], in_=ptab[0:ROWS_PC, :])
            else:
                nc.gpsimd.collective_compute(
                    "ReduceScatter",
                    mybir.AluOpType.add,
                    replica_groups=[list(range(NCORES))],
                    ins=[ptab.opt()],
                    outs=[rsout.opt()],
                )

            # ---- tail ----------------------------------------------------
            rs_s = bigpool.tile([128, PB, TCOLS], f16, tag="rs")
            nc.sync.dma_start(
                out=rs_s[:], in_=rsout.rearrange("(a p) d -> p a d", p=128))
            featp_s = cpool.tile([128, PB, 2 * D], f16, tag="featp")
            nc.sync.dma_start(out=featp_s[:], in_=featpD)
            indegp_s = cpool.tile([128, PB, 2], f16, tag="indegp")
            nc.sync.dma_start(out=indegp_s[:], in_=indegpD)

            idf = cpool.tile([128, PB, 2], f32, tag="idf")
            nc.vector.tensor_copy(idf[:], indegp_s[:])
            maskp = cpool.tile([128, PB, 2], f32, tag="maskp")
            nc.vector.tensor_scalar(maskp[:], idf[:], 0.0, None, OP.is_gt)
            cjp = cpool.tile([128, PB, 2], f32, tag="cjp")
            nc.vector.tensor_scalar_max(cjp[:], idf[:], 1.0)
            nc.scalar.activation(cjp[:], cjp[:], AF.Sqrt)
            nc.vector.reciprocal(cjp[:], cjp[:])
            nc.vector.tensor_mul(cjp[:], cjp[:], maskp[:])
            im1 = cpool.tile([128, PB, 2], f32, tag="im1")
            nc.vector.tensor_scalar(im1[:], maskp[:], -1.0, 1.0,
                                    OP.mult, OP.add)

            # h = agg * cj + featp * (1 - mask), in 5-block chunks so the
            # transposes start while later chunks still blend
            hp = bigpool.tile([128, PB, 2 * D], f32, tag="hp")
            tmp = bigpool.tile([128, PB, 2 * D], f32, tag="tmp")
            hT2 = bigpool.tile([2 * D + 1, ROWS_PC], f16, tag="hT2")
            nc.vector.memset(hT2[:, :], 1.0)
            CB = 5
            with tc.tile_pool(name="tp", bufs=4, space="PSUM") as tppool, \
                 tc.tile_pool(name="oc", bufs=3) as ocpool:
                for g0 in range(0, PB, CB):
                    bs = slice(g0, g0 + CB)
                    for par, c0 in ((0, 0), (1, 64)):
                        sl = slice(par * D, par * D + D)
                        nc.vector.tensor_tensor(
                            hp[:, bs, sl], rs_s[:, bs, c0:c0 + D],
                            cjp[:, bs, par:par + 1].to_broadcast(
                                [128, CB, D]), OP.mult)
                        nc.vector.tensor_tensor(
                            tmp[:, bs, sl], featp_s[:, bs, sl],
                            im1[:, bs, par:par + 1].to_broadcast(
                                [128, CB, D]), OP.mult)
                    nc.vector.tensor_add(hp[:, bs, :], hp[:, bs, :],
                                         tmp[:, bs, :])
                    for b in range(g0, g0 + CB):
                        tp = tppool.tile([2 * D, 128], f32, tag="tp")
                        nc.tensor.transpose(tp[:], hp[:, b, :], ident[:])
                        nc.vector.tensor_copy(
                            hT2[0:2 * D, b * 128:(b + 1) * 128], tp[:])

                CH = 512
                nch = (ROWS_PC + CH - 1) // CH
                for i in range(nch):
                    lo = i * CH
                    hi = min(lo + CH, ROWS_PC)
                    po = tppool.tile([2 * D, CH], f32, tag="po")
                    nc.tensor.matmul(po[:, 0:hi - lo], lhsT=wb2_s[:],
                                     rhs=hT2[:, lo:hi], start=True, stop=True)
                    oc = ocpool.tile([2 * D, CH], f32, tag="oc")
                    nc.scalar.activation(oc[:, 0:hi - lo], po[:, 0:hi - lo],
                                         AF.Relu)
                    nc.sync.dma_start(out=outD[:, lo:hi], in_=oc[:, 0:hi - lo])

    nc.compile()
    return nc


# ---------------------------------------------------------------------------

def kernel(features, src, dst, W, b):
    from concourse.bass_utils import run_bass_kernel_spmd

    in_maps, meta = _host_prep(features, src, dst)

    key = tuple(sorted(meta.items()))
    if key not in _CACHE:
        _CACHE[key] = _build_program(meta)
    nc = _CACHE[key]

    Wt = np.asarray(W, np.float32).T          # [48 in, 48 out]
    bv = np.asarray(b, np.float32)
    wb2 = np.zeros((2 * D + 1, 2 * D), np.float16)
    wb2[0:D, 0:D] = Wt
    wb2[D:2 * D, D:2 * D] = Wt
    wb2[2 * D, 0:D] = bv
    wb2[2 * D, D:2 * D] = bv
    ident = np.eye(128, dtype=np.float32)
    for m in in_maps:
        m["wb2"] = wb2
        m["ident"] = ident

    res = run_bass_kernel_spmd(nc, in_maps, core_ids=list(range(NCORES)))
    globals()["LAST_RESULTS"] = res

    out = np.empty((N, D), np.float32)
    for c in range(NCORES):
        o = res.results[c]["out"].astype(np.float32)
        nlo = c * NPC
        out[nlo:nlo + NPC:2] = o[0:D, :PAIRS_L].T
        out[nlo + 1:nlo + NPC:2] = o[D:2 * D, :PAIRS_L].T
    return np.ascontiguousarray(out, dtype=np.float32)


# revision 7
# speedup vs baseline: 1.0244x; 1.0152x over previous
"""GCN layer on 8 TRN2 cores — scatter-add architecture.

Sharding: cores own contiguous SRC ranges (edge/data parallel per the
sharding hint). Each core computes scaled messages X'[src]=feat*ci for its
~200k out-edges and dma_scatter_add's them into a per-core partial
aggregate table in DRAM (fp16 pair-rows of 256 B; even dst nodes at cols
0:48, odd at 64:112). A ReduceScatter (add) over the 8 partial tables
gives each core the summed aggregates for its 6250-dst slice; the tail
(cj scaling, zero-in-degree fallback, 48x48 linear + relu) runs locally.

Message expansion is free for the first WPOS edges per (src, dst-parity):
scatter instruction r reads the X' SBUF tile directly as its token payload
(token position == local node id).  Remaining edges are host-pre-gathered
raw feature rows, scaled by rsqrt(out_deg) on device; rows receiving many
such edges are pre-summed on the vector engine (8-way groups) so each
leftover stream stays large.

dma_scatter_add loses duplicate target rows *within* one instruction
(parallel read-modify-write against a snapshot), so every instruction's
rows are made unique by construction: positional streams evict in-
instruction duplicates to the dense path; dense streams take the k-th
edge of each (row, parity) bucket; group-reduction emits one token per
row.  Instructions on the same parity view are serialized by the
framework's WAW dependency, which makes cross-instruction accumulation
exact; the two parity views are byte-disjoint so their chains overlap.
"""

import numpy as np

N = 50000
E = 1600000
D = 48
NCORES = 8
NPC = 6250              # real nodes per core
NLOC = 6272             # padded local nodes (49 * 128)
BLOCKS = 49
PAIRS_L = 3125          # real local pairs
ROWS_PC = 3200          # table rows per core (incl. junk rows)
TROWS = ROWS_PC * NCORES
TCOLS = 128             # 256 B rows: E@0:48  O@64:112
WPOS = 14               # positional ranks per (node, dst-parity)
NRAW = 2                # raw dense streams per parity (k-th edge of row)
GK = 6                  # group-reduction fan-in
MAXTOK = 7936           # per-instruction descriptor limit (< 8192)

_CACHE = {}


def _wrap_idx(tok):
    """Token list -> wrapped [128, n/16] int16 (16-partition wrap, x8)."""
    n = len(tok)
    assert n % 16 == 0
    w = np.asarray(tok, np.int16).reshape(n // 16, 16).T
    return np.tile(w, (8, 1))


def _pad128(n):
    return int(np.ceil(max(n, 1) / 128) * 128)


def _host_prep(features, src, dst):
    src = np.asarray(src).astype(np.int64)
    dst = np.asarray(dst).astype(np.int64)
    feats = np.asarray(features, dtype=np.float32)

    out_deg = np.bincount(src, minlength=N).astype(np.int64)
    in_deg = np.bincount(dst, minlength=N).astype(np.int64)

    dcore = dst // NPC
    dloc = dst - dcore * NPC
    trow = (dcore * ROWS_PC + (dloc >> 1)).astype(np.int64)
    dpar = (dst & 1).astype(np.int64)
    score = src // NPC
    sloc = src - score * NPC

    # rank of each edge within its (src, dst-parity) bucket
    key = src * 2 + dpar
    order = np.argsort(key, kind="stable")
    ks = key[order]
    runstart = np.r_[0, np.flatnonzero(np.diff(ks)) + 1]
    runid = np.cumsum(np.bincount(runstart, minlength=E))[:E] - 1
    nrank = np.empty(E, np.int64)
    nrank[order] = np.arange(E) - runstart[runid]

    per_core = []
    for c in range(NCORES):
        sel = score == c
        # relabel local src nodes by descending degree so high-rank
        # positional streams can be statically trimmed
        nlo = c * NPC
        degtot = np.zeros(NLOC, np.int64)
        degtot[:NPC] = out_deg[nlo:nlo + NPC]
        perm = np.argsort(-degtot, kind="stable")   # new pos -> old local
        rperm = np.empty(NLOC, np.int64)
        rperm[perm] = np.arange(NLOC)               # old local -> new pos
        e_sl = rperm[sloc[sel]]
        e_row = trow[sel]
        e_par = dpar[sel]
        e_rank = nrank[sel]

        # positional: rank < WPOS.  In-instruction duplicate rows are
        # repaired by moving the edge to another rank that is free at its
        # node (iterative, a few rounds); unresolved edges fall to dense.
        pos_mask = e_rank < WPOS
        a_rank = e_rank.copy()
        rng = np.random.default_rng(1234 + c)
        evict = np.zeros(len(e_sl), bool)
        for rnd in range(6):
            # rank usage bitmap per (node, parity)
            used = np.zeros((2, NLOC, WPOS), bool)
            keep = pos_mask & ~evict
            used[e_par[keep], e_sl[keep], a_rank[keep]] = True
            changed = 0
            for par in range(2):
                for j in range(WPOS):
                    m = keep & (e_par == par) & (a_rank == j)
                    if not m.any():
                        continue
                    idxs = np.flatnonzero(m)
                    rows = e_row[idxs]
                    o = np.argsort(rows, kind="stable")
                    dupmask = np.zeros(len(idxs), bool)
                    dupmask[o[1:]] = rows[o[1:]] == rows[o[:-1]]
                    losers = idxs[dupmask]
                    if not len(losers):
                        continue
                    if rnd == 5:
                        evict[losers] = True
                        continue
                    # move each loser to a random free rank at its node
                    free = ~used[par, e_sl[losers], :]      # [L, WPOS]
                    anyfree = free.any(axis=1)
                    pick = np.argmax(
                        free * rng.random((len(losers), WPOS)), axis=1)
                    mv = anyfree
                    lm = losers[mv]
                    used[par, e_sl[lm], a_rank[lm]] = False
                    a_rank[lm] = pick[mv]
                    used[par, e_sl[lm], a_rank[lm]] = True
                    evict[losers[~anyfree]] = True
                    changed += int(mv.sum())
            if changed == 0:
                break
        e_rank = a_rank
        use_pos = pos_mask & ~evict

        junk_row = c * ROWS_PC + PAIRS_L
        pos_idx = np.full((2, WPOS, NLOC), junk_row, np.int64)
        pi = np.flatnonzero(use_pos)
        pos_idx[e_par[pi], e_rank[pi], e_sl[pi]] = e_row[pi]
        pos_last = np.zeros((2, WPOS), np.int64)
        for par in range(2):
            for j in range(WPOS):
                m = (e_par[pi] == par) & (e_rank[pi] == j)
                pos_last[par, j] = (e_sl[pi][m].max() + 1) if m.any() else 128

        # dense pool: leftover edges, bucketed per (row, parity)
        di = np.flatnonzero(~use_pos)
        d_row = e_row[di]
        d_par = e_par[di]
        d_src = perm[e_sl[di]] + nlo        # original global src ids
        pc = dict(pos_idx=pos_idx, junk=junk_row)
        for par in range(2):
            m = d_par == par
            rows, srcs = d_row[m], d_src[m]
            o = np.argsort(rows, kind="stable")
            rows, srcs = rows[o], srcs[o]
            rs = np.r_[0, np.flatnonzero(np.diff(rows)) + 1]
            rid = np.cumsum(np.bincount(rs, minlength=len(rows)))[:len(rows)] - 1
            kk = np.arange(len(rows)) - rs[rid]        # rank within row
            cnt = np.diff(np.r_[rs, len(rows)])[rid]   # bucket size per edge
            raw_m = cnt <= NRAW
            raws = []
            for k in range(NRAW):
                s = np.flatnonzero(raw_m & (kk == k))
                raws.append(dict(row=rows[s], src=srcs[s]))
            gs = np.flatnonzero(~raw_m)
            grows, gsrcs, gkk = rows[gs], srcs[gs], kk[gs]
            chunk = gkk // GK
            nchunk = int(chunk.max()) + 1 if len(chunk) else 0
            grp = []
            for ch in range(nchunk):
                s = np.flatnonzero(chunk == ch)
                r = grows[s]
                u, inv = np.unique(r, return_inverse=True)
                mem = gkk[s] - ch * GK
                grp.append(dict(rows=u, ginv=inv, mem=mem, src=gsrcs[s]))
            pc[f"raw{par}"] = raws
            pc[f"grp{par}"] = grp
        pc["pos_last"] = pos_last
        pc["perm"] = perm
        per_core.append(pc)

    # uniform sizes across cores
    pos_len = np.zeros((2, WPOS), np.int64)
    for par in range(2):
        for j in range(WPOS):
            pos_len[par, j] = _pad128(max(
                per_core[c]["pos_last"][par, j] for c in range(NCORES)))
    raw_sz = {0: [], 1: []}
    grp_sz = {0: [], 1: []}
    for par in range(2):
        nraw = max(len(per_core[c][f"raw{par}"]) for c in range(NCORES))
        for c in range(NCORES):
            while len(per_core[c][f"raw{par}"]) < nraw:
                per_core[c][f"raw{par}"].append(
                    dict(row=np.zeros(0, np.int64), src=np.zeros(0, np.int64)))
        for k in range(nraw):
            mx = max(len(per_core[c][f"raw{par}"][k]["row"])
                     for c in range(NCORES))
            raw_sz[par].append(_pad128(mx))
        ng = max(len(per_core[c][f"grp{par}"]) for c in range(NCORES))
        for ch in range(ng):
            mx = max((len(per_core[c][f"grp{par}"][ch]["rows"])
                      if ch < len(per_core[c][f"grp{par}"]) else 0)
                     for c in range(NCORES))
            grp_sz[par].append(_pad128(mx))
    # split any oversized raw stream at the instruction cap; pieces of a
    # duplicate-free stream stay duplicate-free
    for par in range(2):
        while True:
            for k, sz in enumerate(raw_sz[par]):
                if sz > MAXTOK:
                    for c in range(NCORES):
                        d = per_core[c][f"raw{par}"][k]
                        cut = min(MAXTOK, len(d["row"]))
                        per_core[c][f"raw{par}"].insert(
                            k + 1,
                            dict(row=d["row"][cut:], src=d["src"][cut:]))
                        d["row"], d["src"] = d["row"][:cut], d["src"][:cut]
                    raw_sz[par][k] = MAXTOK
                    raw_sz[par].insert(k + 1, _pad128(max(
                        len(per_core[c][f"raw{par}"][k + 1]["row"])
                        for c in range(NCORES))))
                    break
            else:
                break

    CDR = sum(raw_sz[0]) + sum(raw_sz[1])        # raw dense tokens
    NGT = sum(grp_sz[0]) + sum(grp_sz[1])        # group tokens (reduced)

    in_maps = []
    for c in range(NCORES):
        pc = per_core[c]
        junk = pc["junk"]

        idx_cols = []
        for par in range(2):
            for j in range(WPOS):
                idx_cols.append(
                    _wrap_idx(pc["pos_idx"][par, j][: pos_len[par, j]]))

        dmsg = np.zeros((CDR, D), np.float32)
        ddeg = np.ones(CDR, np.float64)
        coff = 0
        for par in range(2):
            for k in range(len(raw_sz[par])):
                d = pc[f"raw{par}"][k]
                n = raw_sz[par][k]
                tok = np.full(n, junk, np.int64)
                tok[: len(d["row"])] = d["row"]
                idx_cols.append(_wrap_idx(tok))
                if len(d["src"]):
                    dmsg[coff:coff + len(d["src"])] = feats[d["src"]]
                    ddeg[coff:coff + len(d["src"])] = out_deg[d["src"]]
                coff += n

        # group region: [NGT, D, GK], member index innermost
        gmsg = np.zeros((NGT, D, GK), np.float32)
        gdeg = np.ones((NGT, GK), np.float64)
        goff = 0
        for par in range(2):
            for ch in range(len(grp_sz[par])):
                n = grp_sz[par][ch]
                tok = np.full(n, junk, np.int64)
                if ch < len(pc[f"grp{par}"]):
                    g = pc[f"grp{par}"][ch]
                    tok[: len(g["rows"])] = g["rows"]
                    gi = goff + g["ginv"]
                    gmsg[gi, :, g["mem"]] = feats[g["src"]]
                    gdeg[gi, g["mem"]] = out_deg[g["src"]]
                idx_cols.append(_wrap_idx(tok))
                goff += n
        sidx = np.concatenate(idx_cols, axis=1)

        dmsg_dev = np.ascontiguousarray(
            dmsg.reshape(CDR // 128, 128, D).transpose(1, 0, 2)
        ).astype(np.float16)
        ddeg_dev = np.ascontiguousarray(
            ddeg.reshape(CDR // 128, 128).T).astype(np.float16)
        gmsg_dev = np.ascontiguousarray(
            gmsg.reshape(NGT // 128, 128, D, GK).transpose(1, 0, 2, 3)
        ).astype(np.float16)
        gdeg_dev = np.ascontiguousarray(
            gdeg.reshape(NGT // 128, 128, GK).transpose(1, 0, 2)
        ).astype(np.float16)

        nlo = c * NPC
        perm = pc["perm"]
        fc = np.zeros((NLOC, D), np.float16)
        fc[:NPC] = feats[nlo:nlo + NPC]
        fc = fc[perm]                       # degree-sorted order
        featc = np.ascontiguousarray(
            fc.reshape(BLOCKS, 128, D).transpose(1, 0, 2))
        odg = np.zeros(NLOC, np.float16)
        odg[:NPC] = out_deg[nlo:nlo + NPC]
        odg = odg[perm]
        outdegc = np.ascontiguousarray(odg.reshape(BLOCKS, 128).T)

        fpl = np.zeros((ROWS_PC, 2 * D), np.float16)
        fpl[:PAIRS_L, 0:D] = feats[nlo:nlo + NPC:2]
        fpl[:PAIRS_L, D:2 * D] = feats[nlo + 1:nlo + NPC:2]
        featp = np.ascontiguousarray(
            fpl.reshape(ROWS_PC // 128, 128, 2 * D).transpose(1, 0, 2))
        idg = np.zeros((ROWS_PC, 2), np.float16)
        idg[:PAIRS_L, 0] = in_deg[nlo:nlo + NPC:2]
        idg[:PAIRS_L, 1] = in_deg[nlo + 1:nlo + NPC:2]
        indegp = np.ascontiguousarray(
            idg.reshape(ROWS_PC // 128, 128, 2).transpose(1, 0, 2))

        in_maps.append(dict(sidx=sidx, dmsg=dmsg_dev, ddeg=ddeg_dev,
                            gmsg=gmsg_dev, gdeg=gdeg_dev,
                            featc=featc, outdegc=outdegc,
                            featp=featp, indegp=indegp))

    meta = dict(raw_sz0=tuple(raw_sz[0]), raw_sz1=tuple(raw_sz[1]),
                grp_sz0=tuple(grp_sz[0]), grp_sz1=tuple(grp_sz[1]),
                pos_len=tuple(map(tuple, pos_len.tolist())),
                CDR=CDR, NGT=NGT)
    return in_maps, meta


# ---------------------------------------------------------------------------

def _build_program(meta):
    import os

    import concourse.tile as tile
    from concourse import bacc, mybir

    f16 = mybir.dt.float16
    f32 = mybir.dt.float32
    i16 = mybir.dt.int16
    AF = mybir.ActivationFunctionType
    OP = mybir.AluOpType
    AX = mybir.AxisListType

    raw_sz = {0: meta["raw_sz0"], 1: meta["raw_sz1"]}
    grp_sz = {0: meta["grp_sz0"], 1: meta["grp_sz1"]}
    pos_len = meta["pos_len"]
    CDR, NGT = meta["CDR"], meta["NGT"]
    TOTIDX = sum(pos_len[0]) + sum(pos_len[1]) + CDR + NGT
    PB = ROWS_PC // 128

    nc = bacc.Bacc("TRN2", target_bir_lowering=False, debug=False,
                   num_devices=NCORES, num_swdge_queues=2)

    sidxD = nc.dram_tensor("sidx", [128, TOTIDX // 16], i16,
                           kind="ExternalInput").ap()
    dmsgD = nc.dram_tensor("dmsg", [128, CDR // 128, D], f16,
                           kind="ExternalInput").ap()
    ddegD = nc.dram_tensor("ddeg", [128, CDR // 128], f16,
                           kind="ExternalInput").ap()
    gmsgD = nc.dram_tensor("gmsg", [128, NGT // 128, D, GK], f16,
                           kind="ExternalInput").ap()
    gdegD = nc.dram_tensor("gdeg", [128, NGT // 128, GK], f16,
                           kind="ExternalInput").ap()
    featcD = nc.dram_tensor("featc", [128, BLOCKS, D], f16,
                            kind="ExternalInput").ap()
    outdegcD = nc.dram_tensor("outdegc", [128, BLOCKS], f16,
                              kind="ExternalInput").ap()
    featpD = nc.dram_tensor("featp", [128, ROWS_PC // 128, 2 * D], f16,
                            kind="ExternalInput").ap()
    indegpD = nc.dram_tensor("indegp", [128, ROWS_PC // 128, 2], f16,
                             kind="ExternalInput").ap()
    wb2D = nc.dram_tensor("wb2", [2 * D + 1, 2 * D], f16,
                          kind="ExternalInput").ap()
    identD = nc.dram_tensor("ident", [128, 128], f32,
                            kind="ExternalInput").ap()
    outD = nc.dram_tensor("out", [2 * D, ROWS_PC], f32,
                          kind="ExternalOutput").ap()

    with tile.TileContext(nc) as tc:
        with tc.tile_pool(name="const", bufs=1) as cpool, \
             tc.tile_pool(name="big", bufs=1) as bigpool, \
             tc.tile_pool(name="dram", bufs=1, space="DRAM") as drampool:

            wb2_s = cpool.tile([2 * D + 1, 2 * D], f16, tag="wb2")
            nc.sync.dma_start(out=wb2_s[:], in_=wb2D)
            ident = cpool.tile([128, 128], f32, tag="ident")
            nc.sync.dma_start(out=ident[:], in_=identD)

            # partial table + contiguous zero-init
            ptab = drampool.tile([TROWS, TCOLS], f16)
            ZB = 50
            zer = cpool.tile([128, ZB, TCOLS], f16, tag="zer")
            nc.vector.memset(zer[:], 0.0)
            ptz = ptab.rearrange("(p a) d -> p a d", p=128)
            for z in range(TROWS // 128 // ZB):
                nc.sync.dma_start(out=ptz[:, z * ZB:(z + 1) * ZB, :],
                                  in_=zer[:])

            sidx = bigpool.tile([128, TOTIDX // 16], i16, tag="sidx")
            PC0 = sum(pos_len[0]) // 16          # O-parity idx col offset
            early = 4 * NLOC // 16
            cuts = [(0, early), (PC0, PC0 + early),
                    (early, PC0), (PC0 + early, TOTIDX // 16)]
            for lo, hi in cuts:
                nc.sync.dma_start(out=sidx[:, lo:hi], in_=sidxD[:, lo:hi])

            # X' = featc * rsqrt(max(outdeg,1))
            featc_s = cpool.tile([128, BLOCKS, D], f16, tag="featc")
            nc.sync.dma_start(out=featc_s[:], in_=featcD)
            odeg = cpool.tile([128, BLOCKS], f16, tag="odeg")
            nc.sync.dma_start(out=odeg[:], in_=outdegcD)
            ci = cpool.tile([128, BLOCKS], f32, tag="ci")
            nc.vector.tensor_copy(ci[:], odeg[:])
            nc.vector.tensor_scalar_max(ci[:], ci[:], 1.0)
            nc.scalar.activation(ci[:], ci[:], AF.Sqrt)
            nc.vector.reciprocal(ci[:], ci[:])
            xp = bigpool.tile([128, BLOCKS, D], f16, tag="xp")
            nc.vector.tensor_tensor(
                xp[:], featc_s[:],
                ci[:, :].unsqueeze(2).to_broadcast([128, BLOCKS, D]),
                OP.mult)

            # ---- scatter chains (one per dst parity) ---------------------
            voff = {0: 0, 1: 64}
            state = dict(icol=0, qn=0)

            def scat(n_tok, in_slice, par):
                c0 = voff[par]
                nc.gpsimd.dma_scatter_add(
                    out_ap=ptab[:, c0:c0 + D],
                    in_ap=in_slice,
                    idxs_ap=sidx[:, state["icol"]:state["icol"] + n_tok // 16],
                    num_idxs=n_tok,
                    num_idxs_reg=n_tok,
                    elem_size=D,
                    elem_step=TCOLS,
                    queue_num=state["qn"] % 2,
                    single_packet=False,
                )
                state["icol"] += n_tok // 16
                state["qn"] += 1

            for par in range(2):
                for j in range(WPOS):
                    n = pos_len[par][j]
                    scat(n, xp[:, 0:n // 128, :], par)
            # dense raw messages scaled in place
            dmsg_s = bigpool.tile([128, CDR // 128, D], f16, tag="dmsg")
            nc.sync.dma_start(out=dmsg_s[:], in_=dmsgD)
            ddeg_s = cpool.tile([128, CDR // 128], f16, tag="ddeg")
            nc.sync.dma_start(out=ddeg_s[:], in_=ddegD)
            cid = cpool.tile([128, CDR // 128], f32, tag="cid")
            nc.vector.tensor_copy(cid[:], ddeg_s[:])
            nc.vector.tensor_scalar_max(cid[:], cid[:], 1.0)
            nc.scalar.activation(cid[:], cid[:], AF.Sqrt)
            nc.vector.reciprocal(cid[:], cid[:])
            nc.vector.tensor_tensor(
                dmsg_s[:], dmsg_s[:],
                cid[:, :].unsqueeze(2).to_broadcast([128, CDR // 128, D]),
                OP.mult)

            # group region: scale members then GK-way reduce
            GC = NGT // 128
            gmsg_s = bigpool.tile([128, GC, D, GK], f16, tag="gmsg")
            nc.sync.dma_start(out=gmsg_s[:], in_=gmsgD)
            gdeg_s = cpool.tile([128, GC, GK], f16, tag="gdeg")
            nc.sync.dma_start(out=gdeg_s[:], in_=gdegD)
            cig = cpool.tile([128, GC, GK], f32, tag="cig")
            nc.vector.tensor_copy(cig[:], gdeg_s[:])
            nc.vector.tensor_scalar_max(cig[:], cig[:], 1.0)
            nc.scalar.activation(cig[:], cig[:], AF.Sqrt)
            nc.vector.reciprocal(cig[:], cig[:])
            nc.vector.tensor_tensor(
                gmsg_s[:], gmsg_s[:],
                cig[:].unsqueeze(2).to_broadcast([128, GC, D, GK]),
                OP.mult)
            gred32 = cpool.tile([128, GC, D], f32, tag="gred32")
            nc.vector.tensor_reduce(gred32[:], gmsg_s[:], AX.X, OP.add)
            gred = cpool.tile([128, GC, D], f16, tag="gred")
            nc.vector.tensor_copy(gred[:], gred32[:])

            dcol = 0
            for par in range(2):
                for k in range(len(raw_sz[par])):
                    n = raw_sz[par][k]
                    scat(n, dmsg_s[:, dcol:dcol + n // 128, :], par)
                    dcol += n // 128
            gcol = 0
            for par in range(2):
                for ch in range(len(grp_sz[par])):
                    n = grp_sz[par][ch]
                    scat(n, gred[:, gcol:gcol + n // 128, :], par)
                    gcol += n // 128

            # ---- ReduceScatter -------------------------------------------
            rsout = drampool.tile([ROWS_PC, TCOLS], f16)
            if os.environ.get("V2_SKIP_RS"):
                nc.gpsimd.dma_start(out=rsout[:], in_=ptab[0:ROWS_PC, :])
            else:
                nc.gpsimd.collective_compute(
                    "ReduceScatter",
                    mybir.AluOpType.add,
                    replica_groups=[list(range(NCORES))],
                    ins=[ptab.opt()],
                    outs=[rsout.opt()],
                )

            # ---- tail ----------------------------------------------------
            rs_s = bigpool.tile([128, PB, TCOLS], f16, tag="rs")
            rsv = rsout.rearrange("(a p) d -> p a d", p=128)
            for g0 in range(0, PB, 5):
                nc.sync.dma_start(out=rs_s[:, g0:g0 + 5, :],
                                  in_=rsv[:, g0:g0 + 5, :])
            featp_s = cpool.tile([128, PB, 2 * D], f16, tag="featp")
            nc.sync.dma_start(out=featp_s[:], in_=featpD)
            indegp_s = cpool.tile([128, PB, 2], f16, tag="indegp")
            nc.sync.dma_start(out=indegp_s[:], in_=indegpD)

            idf = cpool.tile([128, PB, 2], f32, tag="idf")
            nc.vector.tensor_copy(idf[:], indegp_s[:])
            maskp = cpool.tile([128, PB, 2], f32, tag="maskp")
            nc.vector.tensor_scalar(maskp[:], idf[:], 0.0, None, OP.is_gt)
            cjp = cpool.tile([128, PB, 2], f32, tag="cjp")
            nc.vector.tensor_scalar_max(cjp[:], idf[:], 1.0)
            nc.scalar.activation(cjp[:], cjp[:], AF.Sqrt)
            nc.vector.reciprocal(cjp[:], cjp[:])
            nc.vector.tensor_mul(cjp[:], cjp[:], maskp[:])
            im1 = cpool.tile([128, PB, 2], f32, tag="im1")
            nc.vector.tensor_scalar(im1[:], maskp[:], -1.0, 1.0,
                                    OP.mult, OP.add)

            # h = agg * cj + featp * (1 - mask), in 5-block chunks so the
            # transposes start while later chunks still blend
            hp = bigpool.tile([128, PB, 2 * D], f32, tag="hp")
            tmp = bigpool.tile([128, PB, 2 * D], f32, tag="tmp")
            hT2 = bigpool.tile([2 * D + 1, ROWS_PC], f16, tag="hT2")
            nc.vector.memset(hT2[:, :], 1.0)
            CB = 5
            with tc.tile_pool(name="tp", bufs=4, space="PSUM") as tppool, \
                 tc.tile_pool(name="oc", bufs=3) as ocpool:
                for g0 in range(0, PB, CB):
                    bs = slice(g0, g0 + CB)
                    for par, c0 in ((0, 0), (1, 64)):
                        sl = slice(par * D, par * D + D)
                        nc.vector.tensor_tensor(
                            hp[:, bs, sl], rs_s[:, bs, c0:c0 + D],
                            cjp[:, bs, par:par + 1].to_broadcast(
                                [128, CB, D]), OP.mult)
                        nc.vector.tensor_tensor(
                            tmp[:, bs, sl], featp_s[:, bs, sl],
                            im1[:, bs, par:par + 1].to_broadcast(
                                [128, CB, D]), OP.mult)
                    nc.vector.tensor_add(hp[:, bs, :], hp[:, bs, :],
                                         tmp[:, bs, :])
                    for b in range(g0, g0 + CB):
                        tp = tppool.tile([2 * D, 128], f32, tag="tp")
                        nc.tensor.transpose(tp[:], hp[:, b, :], ident[:])
                        nc.vector.tensor_copy(
                            hT2[0:2 * D, b * 128:(b + 1) * 128], tp[:])

                CH = 512
                nch = (ROWS_PC + CH - 1) // CH
                for i in range(nch):
                    lo = i * CH
                    hi = min(lo + CH, ROWS_PC)
                    po = tppool.tile([2 * D, CH], f32, tag="po")
                    nc.tensor.matmul(po[:, 0:hi - lo], lhsT=wb2_s[:],
                                     rhs=hT2[:, lo:hi], start=True, stop=True)
                    oc = ocpool.tile([2 * D, CH], f32, tag="oc")
                    nc.scalar.activation(oc[:, 0:hi - lo], po[:, 0:hi - lo],
                                         AF.Relu)
                    nc.sync.dma_start(out=outD[:, lo:hi], in_=oc[:, 0:hi - lo])

    nc.compile()
    return nc


# ---------------------------------------------------------------------------

def kernel(features, src, dst, W, b):
    from concourse.bass_utils import run_bass_kernel_spmd

    in_maps, meta = _host_prep(features, src, dst)

    key = tuple(sorted(meta.items()))
    if key not in _CACHE:
        _CACHE[key] = _build_program(meta)
    nc = _CACHE[key]

    Wt = np.asarray(W, np.float32).T          # [48 in, 48 out]
    bv = np.asarray(b, np.float32)
    wb2 = np.zeros((2 * D + 1, 2 * D), np.float16)
    wb2[0:D, 0:D] = Wt
    wb2[D:2 * D, D:2 * D] = Wt
    wb2[2 * D, 0:D] = bv
    wb2[2 * D, D:2 * D] = bv
    ident = np.eye(128, dtype=np.float32)
    for m in in_maps:
        m["wb2"] = wb2
        m["ident"] = ident

    res = run_bass_kernel_spmd(nc, in_maps, core_ids=list(range(NCORES)))
    globals()["LAST_RESULTS"] = res

    out = np.empty((N, D), np.float32)
    for c in range(NCORES):
        o = res.results[c]["out"].astype(np.float32)
        nlo = c * NPC
        out[nlo:nlo + NPC:2] = o[0:D, :PAIRS_L].T
        out[nlo + 1:nlo + NPC:2] = o[D:2 * D, :PAIRS_L].T
    return np.ascontiguousarray(out, dtype=np.float32)


# revision 8
# speedup vs baseline: 1.0339x; 1.0093x over previous
"""GCN layer on 8 TRN2 cores — scatter-add architecture.

Sharding: cores own contiguous SRC ranges (edge/data parallel per the
sharding hint). Each core computes scaled messages X'[src]=feat*ci for its
~200k out-edges and dma_scatter_add's them into a per-core partial
aggregate table in DRAM (fp16 pair-rows of 256 B; even dst nodes at cols
0:48, odd at 64:112). A ReduceScatter (add) over the 8 partial tables
gives each core the summed aggregates for its 6250-dst slice; the tail
(cj scaling, zero-in-degree fallback, 48x48 linear + relu) runs locally.

Message expansion is free for the first WPOS edges per (src, dst-parity):
scatter instruction r reads the X' SBUF tile directly as its token payload
(token position == local node id).  Remaining edges are host-pre-gathered
raw feature rows, scaled by rsqrt(out_deg) on device; rows receiving many
such edges are pre-summed on the vector engine (8-way groups) so each
leftover stream stays large.

dma_scatter_add loses duplicate target rows *within* one instruction
(parallel read-modify-write against a snapshot), so every instruction's
rows are made unique by construction: positional streams evict in-
instruction duplicates to the dense path; dense streams take the k-th
edge of each (row, parity) bucket; group-reduction emits one token per
row.  Instructions on the same parity view are serialized by the
framework's WAW dependency, which makes cross-instruction accumulation
exact; the two parity views are byte-disjoint so their chains overlap.
"""

import numpy as np

N = 50000
E = 1600000
D = 48
NCORES = 8
NPC = 6250              # real nodes per core
NLOC = 6272             # padded local nodes (49 * 128)
BLOCKS = 49
PAIRS_L = 3125          # real local pairs
ROWS_PC = 3200          # table rows per core (incl. junk rows)
TROWS = ROWS_PC * NCORES
TCOLS = 128             # 256 B rows: E@0:48  O@64:112
WPOS = 13               # positional ranks per (node, dst-parity)
NRAW = 2                # raw dense streams per parity (k-th edge of row)
GK = 6                  # group-reduction fan-in
MAXTOK = 7936           # per-instruction descriptor limit (< 8192)

_CACHE = {}


def _wrap_idx(tok):
    """Token list -> wrapped [128, n/16] int16 (16-partition wrap, x8)."""
    n = len(tok)
    assert n % 16 == 0
    w = np.asarray(tok, np.int16).reshape(n // 16, 16).T
    return np.tile(w, (8, 1))


def _pad128(n):
    return int(np.ceil(max(n, 1) / 128) * 128)


def _host_prep(features, src, dst):
    src = np.asarray(src).astype(np.int64)
    dst = np.asarray(dst).astype(np.int64)
    feats = np.asarray(features, dtype=np.float32)

    out_deg = np.bincount(src, minlength=N).astype(np.int64)
    in_deg = np.bincount(dst, minlength=N).astype(np.int64)

    dcore = dst // NPC
    dloc = dst - dcore * NPC
    trow = (dcore * ROWS_PC + (dloc >> 1)).astype(np.int64)
    dpar = (dst & 1).astype(np.int64)
    score = src // NPC
    sloc = src - score * NPC

    # rank of each edge within its (src, dst-parity) bucket
    key = src * 2 + dpar
    order = np.argsort(key, kind="stable")
    ks = key[order]
    runstart = np.r_[0, np.flatnonzero(np.diff(ks)) + 1]
    runid = np.cumsum(np.bincount(runstart, minlength=E))[:E] - 1
    nrank = np.empty(E, np.int64)
    nrank[order] = np.arange(E) - runstart[runid]

    per_core = []
    for c in range(NCORES):
        sel = score == c
        # relabel local src nodes by descending degree so high-rank
        # positional streams can be statically trimmed
        nlo = c * NPC
        degtot = np.zeros(NLOC, np.int64)
        degtot[:NPC] = out_deg[nlo:nlo + NPC]
        perm = np.argsort(-degtot, kind="stable")   # new pos -> old local
        rperm = np.empty(NLOC, np.int64)
        rperm[perm] = np.arange(NLOC)               # old local -> new pos
        e_sl = rperm[sloc[sel]]
        e_row = trow[sel]
        e_par = dpar[sel]
        e_rank = nrank[sel]

        # positional: rank < WPOS.  In-instruction duplicate rows are
        # repaired by moving the edge to another rank that is free at its
        # node (iterative, a few rounds); unresolved edges fall to dense.
        pos_mask = e_rank < WPOS
        a_rank = e_rank.copy()
        rng = np.random.default_rng(1234 + c)
        evict = np.zeros(len(e_sl), bool)
        for rnd in range(6):
            # rank usage bitmap per (node, parity)
            used = np.zeros((2, NLOC, WPOS), bool)
            keep = pos_mask & ~evict
            used[e_par[keep], e_sl[keep], a_rank[keep]] = True
            changed = 0
            for par in range(2):
                for j in range(WPOS):
                    m = keep & (e_par == par) & (a_rank == j)
                    if not m.any():
                        continue
                    idxs = np.flatnonzero(m)
                    rows = e_row[idxs]
                    o = np.argsort(rows, kind="stable")
                    dupmask = np.zeros(len(idxs), bool)
                    dupmask[o[1:]] = rows[o[1:]] == rows[o[:-1]]
                    losers = idxs[dupmask]
                    if not len(losers):
                        continue
                    if rnd == 5:
                        evict[losers] = True
                        continue
                    # move each loser to a random free rank at its node
                    free = ~used[par, e_sl[losers], :]      # [L, WPOS]
                    anyfree = free.any(axis=1)
                    pick = np.argmax(
                        free * rng.random((len(losers), WPOS)), axis=1)
                    mv = anyfree
                    lm = losers[mv]
                    used[par, e_sl[lm], a_rank[lm]] = False
                    a_rank[lm] = pick[mv]
                    used[par, e_sl[lm], a_rank[lm]] = True
                    evict[losers[~anyfree]] = True
                    changed += int(mv.sum())
            if changed == 0:
                break
        e_rank = a_rank
        use_pos = pos_mask & ~evict

        junk_row = c * ROWS_PC + PAIRS_L
        pos_idx = np.full((2, WPOS, NLOC), junk_row, np.int64)
        pi = np.flatnonzero(use_pos)
        pos_idx[e_par[pi], e_rank[pi], e_sl[pi]] = e_row[pi]
        pos_last = np.zeros((2, WPOS), np.int64)
        for par in range(2):
            for j in range(WPOS):
                m = (e_par[pi] == par) & (e_rank[pi] == j)
                pos_last[par, j] = (e_sl[pi][m].max() + 1) if m.any() else 128

        # dense pool: leftover edges, bucketed per (row, parity)
        di = np.flatnonzero(~use_pos)
        d_row = e_row[di]
        d_par = e_par[di]
        d_src = perm[e_sl[di]] + nlo        # original global src ids
        pc = dict(pos_idx=pos_idx, junk=junk_row)
        for par in range(2):
            m = d_par == par
            rows, srcs = d_row[m], d_src[m]
            o = np.argsort(rows, kind="stable")
            rows, srcs = rows[o], srcs[o]
            rs = np.r_[0, np.flatnonzero(np.diff(rows)) + 1]
            rid = np.cumsum(np.bincount(rs, minlength=len(rows)))[:len(rows)] - 1
            kk = np.arange(len(rows)) - rs[rid]        # rank within row
            cnt = np.diff(np.r_[rs, len(rows)])[rid]   # bucket size per edge
            raw_m = cnt <= NRAW
            raws = []
            for k in range(NRAW):
                s = np.flatnonzero(raw_m & (kk == k))
                raws.append(dict(row=rows[s], src=srcs[s]))
            gs = np.flatnonzero(~raw_m)
            grows, gsrcs, gkk = rows[gs], srcs[gs], kk[gs]
            chunk = gkk // GK
            nchunk = int(chunk.max()) + 1 if len(chunk) else 0
            grp = []
            for ch in range(nchunk):
                s = np.flatnonzero(chunk == ch)
                r = grows[s]
                u, inv = np.unique(r, return_inverse=True)
                mem = gkk[s] - ch * GK
                grp.append(dict(rows=u, ginv=inv, mem=mem, src=gsrcs[s]))
            pc[f"raw{par}"] = raws
            pc[f"grp{par}"] = grp
        pc["pos_last"] = pos_last
        pc["perm"] = perm
        per_core.append(pc)

    # uniform sizes across cores
    pos_len = np.zeros((2, WPOS), np.int64)
    for par in range(2):
        for j in range(WPOS):
            pos_len[par, j] = _pad128(max(
                per_core[c]["pos_last"][par, j] for c in range(NCORES)))
    raw_sz = {0: [], 1: []}
    grp_sz = {0: [], 1: []}
    for par in range(2):
        nraw = max(len(per_core[c][f"raw{par}"]) for c in range(NCORES))
        for c in range(NCORES):
            while len(per_core[c][f"raw{par}"]) < nraw:
                per_core[c][f"raw{par}"].append(
                    dict(row=np.zeros(0, np.int64), src=np.zeros(0, np.int64)))
        for k in range(nraw):
            mx = max(len(per_core[c][f"raw{par}"][k]["row"])
                     for c in range(NCORES))
            raw_sz[par].append(_pad128(mx))
        ng = max(len(per_core[c][f"grp{par}"]) for c in range(NCORES))
        for ch in range(ng):
            mx = max((len(per_core[c][f"grp{par}"][ch]["rows"])
                      if ch < len(per_core[c][f"grp{par}"]) else 0)
                     for c in range(NCORES))
            grp_sz[par].append(_pad128(mx))
    # split any oversized raw stream at the instruction cap; pieces of a
    # duplicate-free stream stay duplicate-free
    for par in range(2):
        while True:
            for k, sz in enumerate(raw_sz[par]):
                if sz > MAXTOK:
                    for c in range(NCORES):
                        d = per_core[c][f"raw{par}"][k]
                        cut = min(MAXTOK, len(d["row"]))
                        per_core[c][f"raw{par}"].insert(
                            k + 1,
                            dict(row=d["row"][cut:], src=d["src"][cut:]))
                        d["row"], d["src"] = d["row"][:cut], d["src"][:cut]
                    raw_sz[par][k] = MAXTOK
                    raw_sz[par].insert(k + 1, _pad128(max(
                        len(per_core[c][f"raw{par}"][k + 1]["row"])
                        for c in range(NCORES))))
                    break
            else:
                break

    CDR = sum(raw_sz[0]) + sum(raw_sz[1])        # raw dense tokens
    NGT = sum(grp_sz[0]) + sum(grp_sz[1])        # group tokens (reduced)

    in_maps = []
    for c in range(NCORES):
        pc = per_core[c]
        junk = pc["junk"]

        idx_cols = []
        for par in range(2):
            for j in range(WPOS):
                idx_cols.append(
                    _wrap_idx(pc["pos_idx"][par, j][: pos_len[par, j]]))

        dmsg = np.zeros((CDR, D), np.float32)
        ddeg = np.ones(CDR, np.float64)
        coff = 0
        for par in range(2):
            for k in range(len(raw_sz[par])):
                d = pc[f"raw{par}"][k]
                n = raw_sz[par][k]
                tok = np.full(n, junk, np.int64)
                tok[: len(d["row"])] = d["row"]
                idx_cols.append(_wrap_idx(tok))
                if len(d["src"]):
                    dmsg[coff:coff + len(d["src"])] = feats[d["src"]]
                    ddeg[coff:coff + len(d["src"])] = out_deg[d["src"]]
                coff += n

        # group region: [NGT, D, GK], member index innermost
        gmsg = np.zeros((NGT, D, GK), np.float32)
        gdeg = np.ones((NGT, GK), np.float64)
        goff = 0
        for par in range(2):
            for ch in range(len(grp_sz[par])):
                n = grp_sz[par][ch]
                tok = np.full(n, junk, np.int64)
                if ch < len(pc[f"grp{par}"]):
                    g = pc[f"grp{par}"][ch]
                    tok[: len(g["rows"])] = g["rows"]
                    gi = goff + g["ginv"]
                    gmsg[gi, :, g["mem"]] = feats[g["src"]]
                    gdeg[gi, g["mem"]] = out_deg[g["src"]]
                idx_cols.append(_wrap_idx(tok))
                goff += n
        sidx = np.concatenate(idx_cols, axis=1)

        dmsg_dev = np.ascontiguousarray(
            dmsg.reshape(CDR // 128, 128, D).transpose(1, 0, 2)
        ).astype(np.float16)
        ddeg_dev = np.ascontiguousarray(
            ddeg.reshape(CDR // 128, 128).T).astype(np.float16)
        gmsg_dev = np.ascontiguousarray(
            gmsg.reshape(NGT // 128, 128, D, GK).transpose(1, 0, 2, 3)
        ).astype(np.float16)
        gdeg_dev = np.ascontiguousarray(
            gdeg.reshape(NGT // 128, 128, GK).transpose(1, 0, 2)
        ).astype(np.float16)

        nlo = c * NPC
        perm = pc["perm"]
        fc = np.zeros((NLOC, D), np.float16)
        fc[:NPC] = feats[nlo:nlo + NPC]
        fc = fc[perm]                       # degree-sorted order
        featc = np.ascontiguousarray(
            fc.reshape(BLOCKS, 128, D).transpose(1, 0, 2))
        odg = np.zeros(NLOC, np.float16)
        odg[:NPC] = out_deg[nlo:nlo + NPC]
        odg = odg[perm]
        outdegc = np.ascontiguousarray(odg.reshape(BLOCKS, 128).T)

        fpl = np.zeros((ROWS_PC, 2 * D), np.float16)
        fpl[:PAIRS_L, 0:D] = feats[nlo:nlo + NPC:2]
        fpl[:PAIRS_L, D:2 * D] = feats[nlo + 1:nlo + NPC:2]
        featp = np.ascontiguousarray(
            fpl.reshape(ROWS_PC // 128, 128, 2 * D).transpose(1, 0, 2))
        idg = np.zeros((ROWS_PC, 2), np.float16)
        idg[:PAIRS_L, 0] = in_deg[nlo:nlo + NPC:2]
        idg[:PAIRS_L, 1] = in_deg[nlo + 1:nlo + NPC:2]
        indegp = np.ascontiguousarray(
            idg.reshape(ROWS_PC // 128, 128, 2).transpose(1, 0, 2))

        in_maps.append(dict(sidx=sidx, dmsg=dmsg_dev, ddeg=ddeg_dev,
                            gmsg=gmsg_dev, gdeg=gdeg_dev,
                            featc=featc, outdegc=outdegc,
                            featp=featp, indegp=indegp))

    meta = dict(raw_sz0=tuple(raw_sz[0]), raw_sz1=tuple(raw_sz[1]),
                grp_sz0=tuple(grp_sz[0]), grp_sz1=tuple(grp_sz[1]),
                pos_len=tuple(map(tuple, pos_len.tolist())),
                CDR=CDR, NGT=NGT)
    return in_maps, meta


# ---------------------------------------------------------------------------

def _build_program(meta):
    import os

    import concourse.tile as tile
    from concourse import bacc, mybir

    f16 = mybir.dt.float16
    f32 = mybir.dt.float32
    i16 = mybir.dt.int16
    AF = mybir.ActivationFunctionType
    OP = mybir.AluOpType
    AX = mybir.AxisListType

    raw_sz = {0: meta["raw_sz0"], 1: meta["raw_sz1"]}
    grp_sz = {0: meta["grp_sz0"], 1: meta["grp_sz1"]}
    pos_len = meta["pos_len"]
    CDR, NGT = meta["CDR"], meta["NGT"]
    TOTIDX = sum(pos_len[0]) + sum(pos_len[1]) + CDR + NGT
    PB = ROWS_PC // 128

    nc = bacc.Bacc("TRN2", target_bir_lowering=False, debug=False,
                   num_devices=NCORES, num_swdge_queues=2)

    sidxD = nc.dram_tensor("sidx", [128, TOTIDX // 16], i16,
                           kind="ExternalInput").ap()
    dmsgD = nc.dram_tensor("dmsg", [128, CDR // 128, D], f16,
                           kind="ExternalInput").ap()
    ddegD = nc.dram_tensor("ddeg", [128, CDR // 128], f16,
                           kind="ExternalInput").ap()
    gmsgD = nc.dram_tensor("gmsg", [128, NGT // 128, D, GK], f16,
                           kind="ExternalInput").ap()
    gdegD = nc.dram_tensor("gdeg", [128, NGT // 128, GK], f16,
                           kind="ExternalInput").ap()
    featcD = nc.dram_tensor("featc", [128, BLOCKS, D], f16,
                            kind="ExternalInput").ap()
    outdegcD = nc.dram_tensor("outdegc", [128, BLOCKS], f16,
                              kind="ExternalInput").ap()
    featpD = nc.dram_tensor("featp", [128, ROWS_PC // 128, 2 * D], f16,
                            kind="ExternalInput").ap()
    indegpD = nc.dram_tensor("indegp", [128, ROWS_PC // 128, 2], f16,
                             kind="ExternalInput").ap()
    wb2D = nc.dram_tensor("wb2", [2 * D + 1, 2 * D], f16,
                          kind="ExternalInput").ap()
    identD = nc.dram_tensor("ident", [128, 128], f32,
                            kind="ExternalInput").ap()
    outD = nc.dram_tensor("out", [2 * D, ROWS_PC], f32,
                          kind="ExternalOutput").ap()

    with tile.TileContext(nc) as tc:
        with tc.tile_pool(name="const", bufs=1) as cpool, \
             tc.tile_pool(name="big", bufs=1) as bigpool, \
             tc.tile_pool(name="dram", bufs=1, space="DRAM") as drampool:

            wb2_s = cpool.tile([2 * D + 1, 2 * D], f16, tag="wb2")
            nc.sync.dma_start(out=wb2_s[:], in_=wb2D)
            ident = cpool.tile([128, 128], f32, tag="ident")
            nc.sync.dma_start(out=ident[:], in_=identD)

            # partial table + contiguous zero-init
            ptab = drampool.tile([TROWS, TCOLS], f16)
            ZB = 50
            zer = cpool.tile([128, ZB, TCOLS], f16, tag="zer")
            nc.vector.memset(zer[:], 0.0)
            ptz = ptab.rearrange("(p a) d -> p a d", p=128)
            for z in range(TROWS // 128 // ZB):
                nc.sync.dma_start(out=ptz[:, z * ZB:(z + 1) * ZB, :],
                                  in_=zer[:])

            sidx = bigpool.tile([128, TOTIDX // 16], i16, tag="sidx")
            PC0 = sum(pos_len[0]) // 16          # O-parity idx col offset
            early = 4 * NLOC // 16
            cuts = [(0, early), (PC0, PC0 + early),
                    (early, PC0), (PC0 + early, TOTIDX // 16)]
            for lo, hi in cuts:
                nc.sync.dma_start(out=sidx[:, lo:hi], in_=sidxD[:, lo:hi])

            # X' = featc * rsqrt(max(outdeg,1))
            featc_s = cpool.tile([128, BLOCKS, D], f16, tag="featc")
            nc.sync.dma_start(out=featc_s[:], in_=featcD)
            odeg = cpool.tile([128, BLOCKS], f16, tag="odeg")
            nc.sync.dma_start(out=odeg[:], in_=outdegcD)
            ci = cpool.tile([128, BLOCKS], f32, tag="ci")
            nc.vector.tensor_copy(ci[:], odeg[:])
            nc.vector.tensor_scalar_max(ci[:], ci[:], 1.0)
            nc.scalar.activation(ci[:], ci[:], AF.Sqrt)
            nc.vector.reciprocal(ci[:], ci[:])
            xp = bigpool.tile([128, BLOCKS, D], f16, tag="xp")
            nc.vector.tensor_tensor(
                xp[:], featc_s[:],
                ci[:, :].unsqueeze(2).to_broadcast([128, BLOCKS, D]),
                OP.mult)

            # ---- scatter chains (one per dst parity) ---------------------
            voff = {0: 0, 1: 64}
            state = dict(icol=0, qn=0)

            def scat(n_tok, in_slice, par):
                c0 = voff[par]
                nc.gpsimd.dma_scatter_add(
                    out_ap=ptab[:, c0:c0 + D],
                    in_ap=in_slice,
                    idxs_ap=sidx[:, state["icol"]:state["icol"] + n_tok // 16],
                    num_idxs=n_tok,
                    num_idxs_reg=n_tok,
                    elem_size=D,
                    elem_step=TCOLS,
                    queue_num=state["qn"] % 2,
                    single_packet=False,
                )
                state["icol"] += n_tok // 16
                state["qn"] += 1

            for par in range(2):
                for j in range(WPOS):
                    n = pos_len[par][j]
                    scat(n, xp[:, 0:n // 128, :], par)
            # dense raw messages scaled in place
            dmsg_s = bigpool.tile([128, CDR // 128, D], f16, tag="dmsg")
            nc.sync.dma_start(out=dmsg_s[:], in_=dmsgD)
            ddeg_s = cpool.tile([128, CDR // 128], f16, tag="ddeg")
            nc.sync.dma_start(out=ddeg_s[:], in_=ddegD)
            cid = cpool.tile([128, CDR // 128], f32, tag="cid")
            nc.vector.tensor_copy(cid[:], ddeg_s[:])
            nc.vector.tensor_scalar_max(cid[:], cid[:], 1.0)
            nc.scalar.activation(cid[:], cid[:], AF.Sqrt)
            nc.vector.reciprocal(cid[:], cid[:])
            nc.vector.tensor_tensor(
                dmsg_s[:], dmsg_s[:],
                cid[:, :].unsqueeze(2).to_broadcast([128, CDR // 128, D]),
                OP.mult)

            # group region: scale members then GK-way reduce
            GC = NGT // 128
            gmsg_s = bigpool.tile([128, GC, D, GK], f16, tag="gmsg")
            nc.sync.dma_start(out=gmsg_s[:], in_=gmsgD)
            gdeg_s = cpool.tile([128, GC, GK], f16, tag="gdeg")
            nc.sync.dma_start(out=gdeg_s[:], in_=gdegD)
            cig = cpool.tile([128, GC, GK], f32, tag="cig")
            nc.vector.tensor_copy(cig[:], gdeg_s[:])
            nc.vector.tensor_scalar_max(cig[:], cig[:], 1.0)
            nc.scalar.activation(cig[:], cig[:], AF.Sqrt)
            nc.vector.reciprocal(cig[:], cig[:])
            nc.vector.tensor_tensor(
                gmsg_s[:], gmsg_s[:],
                cig[:].unsqueeze(2).to_broadcast([128, GC, D, GK]),
                OP.mult)
            gred32 = cpool.tile([128, GC, D], f32, tag="gred32")
            nc.vector.tensor_reduce(gred32[:], gmsg_s[:], AX.X, OP.add)
            gred = cpool.tile([128, GC, D], f16, tag="gred")
            nc.vector.tensor_copy(gred[:], gred32[:])

            dcol = 0
            for par in range(2):
                for k in range(len(raw_sz[par])):
                    n = raw_sz[par][k]
                    scat(n, dmsg_s[:, dcol:dcol + n // 128, :], par)
                    dcol += n // 128
            gcol = 0
            for par in range(2):
                for ch in range(len(grp_sz[par])):
                    n = grp_sz[par][ch]
                    scat(n, gred[:, gcol:gcol + n // 128, :], par)
                    gcol += n // 128

            # ---- ReduceScatter -------------------------------------------
            rsout = drampool.tile([ROWS_PC, TCOLS], f16)
            if os.environ.get("V2_SKIP_RS"):
                nc.gpsimd.dma_start(out=rsout[:], in_=ptab[0:ROWS_PC, :])
            else:
                nc.gpsimd.collective_compute(
                    "ReduceScatter",
                    mybir.AluOpType.add,
                    replica_groups=[list(range(NCORES))],
                    ins=[ptab.opt()],
                    outs=[rsout.opt()],
                )

            # ---- tail ----------------------------------------------------
            rs_s = bigpool.tile([128, PB, TCOLS], f16, tag="rs")
            rsv = rsout.rearrange("(a p) d -> p a d", p=128)
            for g0 in range(0, PB, 5):
                nc.sync.dma_start(out=rs_s[:, g0:g0 + 5, :],
                                  in_=rsv[:, g0:g0 + 5, :])
            featp_s = cpool.tile([128, PB, 2 * D], f16, tag="featp")
            nc.sync.dma_start(out=featp_s[:], in_=featpD)
            indegp_s = cpool.tile([128, PB, 2], f16, tag="indegp")
            nc.sync.dma_start(out=indegp_s[:], in_=indegpD)

            idf = cpool.tile([128, PB, 2], f32, tag="idf")
            nc.vector.tensor_copy(idf[:], indegp_s[:])
            maskp = cpool.tile([128, PB, 2], f32, tag="maskp")
            nc.vector.tensor_scalar(maskp[:], idf[:], 0.0, None, OP.is_gt)
            cjp = cpool.tile([128, PB, 2], f32, tag="cjp")
            nc.vector.tensor_scalar_max(cjp[:], idf[:], 1.0)
            nc.scalar.activation(cjp[:], cjp[:], AF.Sqrt)
            nc.vector.reciprocal(cjp[:], cjp[:])
            nc.vector.tensor_mul(cjp[:], cjp[:], maskp[:])
            im1 = cpool.tile([128, PB, 2], f32, tag="im1")
            nc.vector.tensor_scalar(im1[:], maskp[:], -1.0, 1.0,
                                    OP.mult, OP.add)

            # h = agg * cj + featp * (1 - mask), in 5-block chunks so the
            # transposes start while later chunks still blend
            hp = bigpool.tile([128, PB, 2 * D], f32, tag="hp")
            tmp = bigpool.tile([128, PB, 2 * D], f32, tag="tmp")
            hT2 = bigpool.tile([2 * D + 1, ROWS_PC], f16, tag="hT2")
            nc.vector.memset(hT2[:, :], 1.0)
            CB = 5
            with tc.tile_pool(name="tp", bufs=4, space="PSUM") as tppool, \
                 tc.tile_pool(name="oc", bufs=3) as ocpool:
                for g0 in range(0, PB, CB):
                    bs = slice(g0, g0 + CB)
                    for par, c0 in ((0, 0), (1, 64)):
                        sl = slice(par * D, par * D + D)
                        nc.vector.tensor_tensor(
                            hp[:, bs, sl], rs_s[:, bs, c0:c0 + D],
                            cjp[:, bs, par:par + 1].to_broadcast(
                                [128, CB, D]), OP.mult)
                        nc.vector.tensor_tensor(
                            tmp[:, bs, sl], featp_s[:, bs, sl],
                            im1[:, bs, par:par + 1].to_broadcast(
                                [128, CB, D]), OP.mult)
                    nc.vector.tensor_add(hp[:, bs, :], hp[:, bs, :],
                                         tmp[:, bs, :])
                    for b in range(g0, g0 + CB):
                        tp = tppool.tile([2 * D, 128], f32, tag="tp")
                        nc.tensor.transpose(tp[:], hp[:, b, :], ident[:])
                        nc.vector.tensor_copy(
                            hT2[0:2 * D, b * 128:(b + 1) * 128], tp[:])

                CH = 512
                nch = (ROWS_PC + CH - 1) // CH
                for i in range(nch):
                    lo = i * CH
                    hi = min(lo + CH, ROWS_PC)
                    po = tppool.tile([2 * D, CH], f32, tag="po")
                    nc.tensor.matmul(po[:, 0:hi - lo], lhsT=wb2_s[:],
                                     rhs=hT2[:, lo:hi], start=True, stop=True)
                    oc = ocpool.tile([2 * D, CH], f32, tag="oc")
                    nc.scalar.activation(oc[:, 0:hi - lo], po[:, 0:hi - lo],
                                         AF.Relu)
                    nc.sync.dma_start(out=outD[:, lo:hi], in_=oc[:, 0:hi - lo])

    nc.compile()
    return nc


# ---------------------------------------------------------------------------

def kernel(features, src, dst, W, b):
    from concourse.bass_utils import run_bass_kernel_spmd

    in_maps, meta = _host_prep(features, src, dst)

    key = tuple(sorted(meta.items()))
    if key not in _CACHE:
        _CACHE[key] = _build_program(meta)
    nc = _CACHE[key]

    Wt = np.asarray(W, np.float32).T          # [48 in, 48 out]
    bv = np.asarray(b, np.float32)
    wb2 = np.zeros((2 * D + 1, 2 * D), np.float16)
    wb2[0:D, 0:D] = Wt
    wb2[D:2 * D, D:2 * D] = Wt
    wb2[2 * D, 0:D] = bv
    wb2[2 * D, D:2 * D] = bv
    ident = np.eye(128, dtype=np.float32)
    for m in in_maps:
        m["wb2"] = wb2
        m["ident"] = ident

    res = run_bass_kernel_spmd(nc, in_maps, core_ids=list(range(NCORES)))
    globals()["LAST_RESULTS"] = res

    out = np.empty((N, D), np.float32)
    for c in range(NCORES):
        o = res.results[c]["out"].astype(np.float32)
        nlo = c * NPC
        out[nlo:nlo + NPC:2] = o[0:D, :PAIRS_L].T
        out[nlo + 1:nlo + NPC:2] = o[D:2 * D, :PAIRS_L].T
    return np.ascontiguousarray(out, dtype=np.float32)


# revision 9
# speedup vs baseline: 1.0435x; 1.0093x over previous
"""GCN layer on 8 TRN2 cores — scatter-add architecture.

Sharding: cores own contiguous SRC ranges (edge/data parallel per the
sharding hint). Each core computes scaled messages X'[src]=feat*ci for its
~200k out-edges and dma_scatter_add's them into a per-core partial
aggregate table in DRAM (fp16 pair-rows of 256 B; even dst nodes at cols
0:48, odd at 64:112). A ReduceScatter (add) over the 8 partial tables
gives each core the summed aggregates for its 6250-dst slice; the tail
(cj scaling, zero-in-degree fallback, 48x48 linear + relu) runs locally.

Message expansion is free for the first WPOS edges per (src, dst-parity):
scatter instruction r reads the X' SBUF tile directly as its token payload
(token position == local node id).  Remaining edges are host-pre-gathered
raw feature rows, scaled by rsqrt(out_deg) on device; rows receiving many
such edges are pre-summed on the vector engine (8-way groups) so each
leftover stream stays large.

dma_scatter_add loses duplicate target rows *within* one instruction
(parallel read-modify-write against a snapshot), so every instruction's
rows are made unique by construction: positional streams evict in-
instruction duplicates to the dense path; dense streams take the k-th
edge of each (row, parity) bucket; group-reduction emits one token per
row.  Instructions on the same parity view are serialized by the
framework's WAW dependency, which makes cross-instruction accumulation
exact; the two parity views are byte-disjoint so their chains overlap.
"""

import numpy as np

N = 50000
E = 1600000
D = 48
NCORES = 8
NPC = 6250              # real nodes per core
NLOC = 6272             # padded local nodes (49 * 128)
BLOCKS = 49
PAIRS_L = 3125          # real local pairs
ROWS_PC = 3200          # table rows per core (incl. junk rows)
TROWS = ROWS_PC * NCORES
TCOLS = 128             # 256 B rows: E@0:48  O@64:112
WPOS = 12               # positional ranks per (node, dst-parity)
NRAW = 2                # raw dense streams per parity (k-th edge of row)
GK = 6                  # group-reduction fan-in
MAXTOK = 7936           # per-instruction descriptor limit (< 8192)

_CACHE = {}


def _wrap_idx(tok):
    """Token list -> wrapped [128, n/16] int16 (16-partition wrap, x8)."""
    n = len(tok)
    assert n % 16 == 0
    w = np.asarray(tok, np.int16).reshape(n // 16, 16).T
    return np.tile(w, (8, 1))


def _pad128(n):
    return int(np.ceil(max(n, 1) / 128) * 128)


def _host_prep(features, src, dst):
    src = np.asarray(src).astype(np.int64)
    dst = np.asarray(dst).astype(np.int64)
    feats = np.asarray(features, dtype=np.float32)

    out_deg = np.bincount(src, minlength=N).astype(np.int64)
    in_deg = np.bincount(dst, minlength=N).astype(np.int64)

    dcore = dst // NPC
    dloc = dst - dcore * NPC
    trow = (dcore * ROWS_PC + (dloc >> 1)).astype(np.int64)
    dpar = (dst & 1).astype(np.int64)
    score = src // NPC
    sloc = src - score * NPC

    # rank of each edge within its (src, dst-parity) bucket
    key = src * 2 + dpar
    order = np.argsort(key, kind="stable")
    ks = key[order]
    runstart = np.r_[0, np.flatnonzero(np.diff(ks)) + 1]
    runid = np.cumsum(np.bincount(runstart, minlength=E))[:E] - 1
    nrank = np.empty(E, np.int64)
    nrank[order] = np.arange(E) - runstart[runid]

    per_core = []
    for c in range(NCORES):
        sel = score == c
        # relabel local src nodes by descending degree so high-rank
        # positional streams can be statically trimmed
        nlo = c * NPC
        degtot = np.zeros(NLOC, np.int64)
        degtot[:NPC] = out_deg[nlo:nlo + NPC]
        perm = np.argsort(-degtot, kind="stable")   # new pos -> old local
        rperm = np.empty(NLOC, np.int64)
        rperm[perm] = np.arange(NLOC)               # old local -> new pos
        e_sl = rperm[sloc[sel]]
        e_row = trow[sel]
        e_par = dpar[sel]
        e_rank = nrank[sel]

        # positional: rank < WPOS.  In-instruction duplicate rows are
        # repaired by moving the edge to another rank that is free at its
        # node (iterative, a few rounds); unresolved edges fall to dense.
        pos_mask = e_rank < WPOS
        a_rank = e_rank.copy()
        rng = np.random.default_rng(1234 + c)
        evict = np.zeros(len(e_sl), bool)
        for rnd in range(6):
            # rank usage bitmap per (node, parity)
            used = np.zeros((2, NLOC, WPOS), bool)
            keep = pos_mask & ~evict
            used[e_par[keep], e_sl[keep], a_rank[keep]] = True
            changed = 0
            for par in range(2):
                for j in range(WPOS):
                    m = keep & (e_par == par) & (a_rank == j)
                    if not m.any():
                        continue
                    idxs = np.flatnonzero(m)
                    rows = e_row[idxs]
                    o = np.argsort(rows, kind="stable")
                    dupmask = np.zeros(len(idxs), bool)
                    dupmask[o[1:]] = rows[o[1:]] == rows[o[:-1]]
                    losers = idxs[dupmask]
                    if not len(losers):
                        continue
                    if rnd == 5:
                        evict[losers] = True
                        continue
                    # move each loser to a random free rank at its node
                    free = ~used[par, e_sl[losers], :]      # [L, WPOS]
                    anyfree = free.any(axis=1)
                    pick = np.argmax(
                        free * rng.random((len(losers), WPOS)), axis=1)
                    mv = anyfree
                    lm = losers[mv]
                    used[par, e_sl[lm], a_rank[lm]] = False
                    a_rank[lm] = pick[mv]
                    used[par, e_sl[lm], a_rank[lm]] = True
                    evict[losers[~anyfree]] = True
                    changed += int(mv.sum())
            if changed == 0:
                break
        e_rank = a_rank
        use_pos = pos_mask & ~evict

        junk_row = c * ROWS_PC + PAIRS_L
        pos_idx = np.full((2, WPOS, NLOC), junk_row, np.int64)
        pi = np.flatnonzero(use_pos)
        pos_idx[e_par[pi], e_rank[pi], e_sl[pi]] = e_row[pi]
        pos_last = np.zeros((2, WPOS), np.int64)
        for par in range(2):
            for j in range(WPOS):
                m = (e_par[pi] == par) & (e_rank[pi] == j)
                pos_last[par, j] = (e_sl[pi][m].max() + 1) if m.any() else 128

        # dense pool: leftover edges, bucketed per (row, parity)
        di = np.flatnonzero(~use_pos)
        d_row = e_row[di]
        d_par = e_par[di]
        d_src = perm[e_sl[di]] + nlo        # original global src ids
        pc = dict(pos_idx=pos_idx, junk=junk_row)
        for par in range(2):
            m = d_par == par
            rows, srcs = d_row[m], d_src[m]
            o = np.argsort(rows, kind="stable")
            rows, srcs = rows[o], srcs[o]
            rs = np.r_[0, np.flatnonzero(np.diff(rows)) + 1]
            rid = np.cumsum(np.bincount(rs, minlength=len(rows)))[:len(rows)] - 1
            kk = np.arange(len(rows)) - rs[rid]        # rank within row
            cnt = np.diff(np.r_[rs, len(rows)])[rid]   # bucket size per edge
            raw_m = cnt <= NRAW
            raws = []
            for k in range(NRAW):
                s = np.flatnonzero(raw_m & (kk == k))
                raws.append(dict(row=rows[s], src=srcs[s]))
            gs = np.flatnonzero(~raw_m)
            grows, gsrcs, gkk = rows[gs], srcs[gs], kk[gs]
            chunk = gkk // GK
            nchunk = int(chunk.max()) + 1 if len(chunk) else 0
            grp = []
            for ch in range(nchunk):
                s = np.flatnonzero(chunk == ch)
                r = grows[s]
                u, inv = np.unique(r, return_inverse=True)
                mem = gkk[s] - ch * GK
                grp.append(dict(rows=u, ginv=inv, mem=mem, src=gsrcs[s]))
            pc[f"raw{par}"] = raws
            pc[f"grp{par}"] = grp
        pc["pos_last"] = pos_last
        pc["perm"] = perm
        per_core.append(pc)

    # uniform sizes across cores
    pos_len = np.zeros((2, WPOS), np.int64)
    for par in range(2):
        for j in range(WPOS):
            pos_len[par, j] = _pad128(max(
                per_core[c]["pos_last"][par, j] for c in range(NCORES)))
    raw_sz = {0: [], 1: []}
    grp_sz = {0: [], 1: []}
    for par in range(2):
        nraw = max(len(per_core[c][f"raw{par}"]) for c in range(NCORES))
        for c in range(NCORES):
            while len(per_core[c][f"raw{par}"]) < nraw:
                per_core[c][f"raw{par}"].append(
                    dict(row=np.zeros(0, np.int64), src=np.zeros(0, np.int64)))
        for k in range(nraw):
            mx = max(len(per_core[c][f"raw{par}"][k]["row"])
                     for c in range(NCORES))
            raw_sz[par].append(_pad128(mx))
        ng = max(len(per_core[c][f"grp{par}"]) for c in range(NCORES))
        for ch in range(ng):
            mx = max((len(per_core[c][f"grp{par}"][ch]["rows"])
                      if ch < len(per_core[c][f"grp{par}"]) else 0)
                     for c in range(NCORES))
            grp_sz[par].append(_pad128(mx))
    # split any oversized raw stream at the instruction cap; pieces of a
    # duplicate-free stream stay duplicate-free
    for par in range(2):
        while True:
            for k, sz in enumerate(raw_sz[par]):
                if sz > MAXTOK:
                    for c in range(NCORES):
                        d = per_core[c][f"raw{par}"][k]
                        cut = min(MAXTOK, len(d["row"]))
                        per_core[c][f"raw{par}"].insert(
                            k + 1,
                            dict(row=d["row"][cut:], src=d["src"][cut:]))
                        d["row"], d["src"] = d["row"][:cut], d["src"][:cut]
                    raw_sz[par][k] = MAXTOK
                    raw_sz[par].insert(k + 1, _pad128(max(
                        len(per_core[c][f"raw{par}"][k + 1]["row"])
                        for c in range(NCORES))))
                    break
            else:
                break

    CDR = sum(raw_sz[0]) + sum(raw_sz[1])        # raw dense tokens
    NGT = sum(grp_sz[0]) + sum(grp_sz[1])        # group tokens (reduced)

    in_maps = []
    for c in range(NCORES):
        pc = per_core[c]
        junk = pc["junk"]

        idx_cols = []
        for par in range(2):
            for j in range(WPOS):
                idx_cols.append(
                    _wrap_idx(pc["pos_idx"][par, j][: pos_len[par, j]]))

        dmsg = np.zeros((CDR, D), np.float32)
        ddeg = np.ones(CDR, np.float64)
        coff = 0
        for par in range(2):
            for k in range(len(raw_sz[par])):
                d = pc[f"raw{par}"][k]
                n = raw_sz[par][k]
                tok = np.full(n, junk, np.int64)
                tok[: len(d["row"])] = d["row"]
                idx_cols.append(_wrap_idx(tok))
                if len(d["src"]):
                    dmsg[coff:coff + len(d["src"])] = feats[d["src"]]
                    ddeg[coff:coff + len(d["src"])] = out_deg[d["src"]]
                coff += n

        # group region: [NGT, D, GK], member index innermost
        gmsg = np.zeros((NGT, D, GK), np.float32)
        gdeg = np.ones((NGT, GK), np.float64)
        goff = 0
        for par in range(2):
            for ch in range(len(grp_sz[par])):
                n = grp_sz[par][ch]
                tok = np.full(n, junk, np.int64)
                if ch < len(pc[f"grp{par}"]):
                    g = pc[f"grp{par}"][ch]
                    tok[: len(g["rows"])] = g["rows"]
                    gi = goff + g["ginv"]
                    gmsg[gi, :, g["mem"]] = feats[g["src"]]
                    gdeg[gi, g["mem"]] = out_deg[g["src"]]
                idx_cols.append(_wrap_idx(tok))
                goff += n
        sidx = np.concatenate(idx_cols, axis=1)

        dmsg_dev = np.ascontiguousarray(
            dmsg.reshape(CDR // 128, 128, D).transpose(1, 0, 2)
        ).astype(np.float16)
        ddeg_dev = np.ascontiguousarray(
            ddeg.reshape(CDR // 128, 128).T).astype(np.float16)
        gmsg_dev = np.ascontiguousarray(
            gmsg.reshape(NGT // 128, 128, D, GK).transpose(1, 0, 2, 3)
        ).astype(np.float16)
        gdeg_dev = np.ascontiguousarray(
            gdeg.reshape(NGT // 128, 128, GK).transpose(1, 0, 2)
        ).astype(np.float16)

        nlo = c * NPC
        perm = pc["perm"]
        fc = np.zeros((NLOC, D), np.float16)
        fc[:NPC] = feats[nlo:nlo + NPC]
        fc = fc[perm]                       # degree-sorted order
        featc = np.ascontiguousarray(
            fc.reshape(BLOCKS, 128, D).transpose(1, 0, 2))
        odg = np.zeros(NLOC, np.float16)
        odg[:NPC] = out_deg[nlo:nlo + NPC]
        odg = odg[perm]
        outdegc = np.ascontiguousarray(odg.reshape(BLOCKS, 128).T)

        fpl = np.zeros((ROWS_PC, 2 * D), np.float16)
        fpl[:PAIRS_L, 0:D] = feats[nlo:nlo + NPC:2]
        fpl[:PAIRS_L, D:2 * D] = feats[nlo + 1:nlo + NPC:2]
        featp = np.ascontiguousarray(
            fpl.reshape(ROWS_PC // 128, 128, 2 * D).transpose(1, 0, 2))
        idg = np.zeros((ROWS_PC, 2), np.float16)
        idg[:PAIRS_L, 0] = in_deg[nlo:nlo + NPC:2]
        idg[:PAIRS_L, 1] = in_deg[nlo + 1:nlo + NPC:2]
        indegp = np.ascontiguousarray(
            idg.reshape(ROWS_PC // 128, 128, 2).transpose(1, 0, 2))

        in_maps.append(dict(sidx=sidx, dmsg=dmsg_dev, ddeg=ddeg_dev,
                            gmsg=gmsg_dev, gdeg=gdeg_dev,
                            featc=featc, outdegc=outdegc,
                            featp=featp, indegp=indegp))

    meta = dict(raw_sz0=tuple(raw_sz[0]), raw_sz1=tuple(raw_sz[1]),
                grp_sz0=tuple(grp_sz[0]), grp_sz1=tuple(grp_sz[1]),
                pos_len=tuple(map(tuple, pos_len.tolist())),
                CDR=CDR, NGT=NGT)
    return in_maps, meta


# ---------------------------------------------------------------------------

def _build_program(meta):
    import os

    import concourse.tile as tile
    from concourse import bacc, mybir

    f16 = mybir.dt.float16
    f32 = mybir.dt.float32
    i16 = mybir.dt.int16
    AF = mybir.ActivationFunctionType
    OP = mybir.AluOpType
    AX = mybir.AxisListType

    raw_sz = {0: meta["raw_sz0"], 1: meta["raw_sz1"]}
    grp_sz = {0: meta["grp_sz0"], 1: meta["grp_sz1"]}
    pos_len = meta["pos_len"]
    CDR, NGT = meta["CDR"], meta["NGT"]
    TOTIDX = sum(pos_len[0]) + sum(pos_len[1]) + CDR + NGT
    PB = ROWS_PC // 128

    nc = bacc.Bacc("TRN2", target_bir_lowering=False, debug=False,
                   num_devices=NCORES, num_swdge_queues=2)

    sidxD = nc.dram_tensor("sidx", [128, TOTIDX // 16], i16,
                           kind="ExternalInput").ap()
    dmsgD = nc.dram_tensor("dmsg", [128, CDR // 128, D], f16,
                           kind="ExternalInput").ap()
    ddegD = nc.dram_tensor("ddeg", [128, CDR // 128], f16,
                           kind="ExternalInput").ap()
    gmsgD = nc.dram_tensor("gmsg", [128, NGT // 128, D, GK], f16,
                           kind="ExternalInput").ap()
    gdegD = nc.dram_tensor("gdeg", [128, NGT // 128, GK], f16,
                           kind="ExternalInput").ap()
    featcD = nc.dram_tensor("featc", [128, BLOCKS, D], f16,
                            kind="ExternalInput").ap()
    outdegcD = nc.dram_tensor("outdegc", [128, BLOCKS], f16,
                              kind="ExternalInput").ap()
    featpD = nc.dram_tensor("featp", [128, ROWS_PC // 128, 2 * D], f16,
                            kind="ExternalInput").ap()
    indegpD = nc.dram_tensor("indegp", [128, ROWS_PC // 128, 2], f16,
                             kind="ExternalInput").ap()
    wb2D = nc.dram_tensor("wb2", [2 * D + 1, 2 * D], f16,
                          kind="ExternalInput").ap()
    identD = nc.dram_tensor("ident", [128, 128], f32,
                            kind="ExternalInput").ap()
    outD = nc.dram_tensor("out", [2 * D, ROWS_PC], f32,
                          kind="ExternalOutput").ap()

    with tile.TileContext(nc) as tc:
        with tc.tile_pool(name="const", bufs=1) as cpool, \
             tc.tile_pool(name="big", bufs=1) as bigpool, \
             tc.tile_pool(name="dram", bufs=1, space="DRAM") as drampool:

            wb2_s = cpool.tile([2 * D + 1, 2 * D], f16, tag="wb2")
            nc.sync.dma_start(out=wb2_s[:], in_=wb2D)
            ident = cpool.tile([128, 128], f32, tag="ident")
            nc.sync.dma_start(out=ident[:], in_=identD)

            # partial table + contiguous zero-init
            ptab = drampool.tile([TROWS, TCOLS], f16)
            ZB = 50
            zer = cpool.tile([128, ZB, TCOLS], f16, tag="zer")
            nc.vector.memset(zer[:], 0.0)
            ptz = ptab.rearrange("(p a) d -> p a d", p=128)
            for z in range(TROWS // 128 // ZB):
                nc.sync.dma_start(out=ptz[:, z * ZB:(z + 1) * ZB, :],
                                  in_=zer[:])

            sidx = bigpool.tile([128, TOTIDX // 16], i16, tag="sidx")
            PC0 = sum(pos_len[0]) // 16          # O-parity idx col offset
            early = 4 * NLOC // 16
            cuts = [(0, early), (PC0, PC0 + early),
                    (early, PC0), (PC0 + early, TOTIDX // 16)]
            for lo, hi in cuts:
                nc.sync.dma_start(out=sidx[:, lo:hi], in_=sidxD[:, lo:hi])

            # X' = featc * rsqrt(max(outdeg,1))
            featc_s = cpool.tile([128, BLOCKS, D], f16, tag="featc")
            nc.sync.dma_start(out=featc_s[:], in_=featcD)
            odeg = cpool.tile([128, BLOCKS], f16, tag="odeg")
            nc.sync.dma_start(out=odeg[:], in_=outdegcD)
            ci = cpool.tile([128, BLOCKS], f32, tag="ci")
            nc.vector.tensor_copy(ci[:], odeg[:])
            nc.vector.tensor_scalar_max(ci[:], ci[:], 1.0)
            nc.scalar.activation(ci[:], ci[:], AF.Sqrt)
            nc.vector.reciprocal(ci[:], ci[:])
            xp = bigpool.tile([128, BLOCKS, D], f16, tag="xp")
            nc.vector.tensor_tensor(
                xp[:], featc_s[:],
                ci[:, :].unsqueeze(2).to_broadcast([128, BLOCKS, D]),
                OP.mult)

            # ---- scatter chains (one per dst parity) ---------------------
            voff = {0: 0, 1: 64}
            state = dict(icol=0, qn=0)

            def scat(n_tok, in_slice, par):
                c0 = voff[par]
                nc.gpsimd.dma_scatter_add(
                    out_ap=ptab[:, c0:c0 + D],
                    in_ap=in_slice,
                    idxs_ap=sidx[:, state["icol"]:state["icol"] + n_tok // 16],
                    num_idxs=n_tok,
                    num_idxs_reg=n_tok,
                    elem_size=D,
                    elem_step=TCOLS,
                    queue_num=state["qn"] % 2,
                    single_packet=False,
                )
                state["icol"] += n_tok // 16
                state["qn"] += 1

            for par in range(2):
                for j in range(WPOS):
                    n = pos_len[par][j]
                    scat(n, xp[:, 0:n // 128, :], par)
            # dense raw messages scaled in place
            dmsg_s = bigpool.tile([128, CDR // 128, D], f16, tag="dmsg")
            nc.sync.dma_start(out=dmsg_s[:], in_=dmsgD)
            ddeg_s = cpool.tile([128, CDR // 128], f16, tag="ddeg")
            nc.sync.dma_start(out=ddeg_s[:], in_=ddegD)
            cid = cpool.tile([128, CDR // 128], f32, tag="cid")
            nc.vector.tensor_copy(cid[:], ddeg_s[:])
            nc.vector.tensor_scalar_max(cid[:], cid[:], 1.0)
            nc.scalar.activation(cid[:], cid[:], AF.Sqrt)
            nc.vector.reciprocal(cid[:], cid[:])
            nc.vector.tensor_tensor(
                dmsg_s[:], dmsg_s[:],
                cid[:, :].unsqueeze(2).to_broadcast([128, CDR // 128, D]),
                OP.mult)

            # group region: scale members then GK-way reduce
            GC = NGT // 128
            gmsg_s = bigpool.tile([128, GC, D, GK], f16, tag="gmsg")
            nc.sync.dma_start(out=gmsg_s[:], in_=gmsgD)
            gdeg_s = cpool.tile([128, GC, GK], f16, tag="gdeg")
            nc.sync.dma_start(out=gdeg_s[:], in_=gdegD)
            cig = cpool.tile([128, GC, GK], f32, tag="cig")
            nc.vector.tensor_copy(cig[:], gdeg_s[:])
            nc.vector.tensor_scalar_max(cig[:], cig[:], 1.0)
            nc.scalar.activation(cig[:], cig[:], AF.Sqrt)
            nc.vector.reciprocal(cig[:], cig[:])
            nc.vector.tensor_tensor(
                gmsg_s[:], gmsg_s[:],
                cig[:].unsqueeze(2).to_broadcast([128, GC, D, GK]),
                OP.mult)
            gred32 = cpool.tile([128, GC, D], f32, tag="gred32")
            nc.vector.tensor_reduce(gred32[:], gmsg_s[:], AX.X, OP.add)
            gred = cpool.tile([128, GC, D], f16, tag="gred")
            nc.vector.tensor_copy(gred[:], gred32[:])

            dcol = 0
            for par in range(2):
                for k in range(len(raw_sz[par])):
                    n = raw_sz[par][k]
                    scat(n, dmsg_s[:, dcol:dcol + n // 128, :], par)
                    dcol += n // 128
            gcol = 0
            for par in range(2):
                for ch in range(len(grp_sz[par])):
                    n = grp_sz[par][ch]
                    scat(n, gred[:, gcol:gcol + n // 128, :], par)
                    gcol += n // 128

            # ---- ReduceScatter -------------------------------------------
            rsout = drampool.tile([ROWS_PC, TCOLS], f16)
            if os.environ.get("V2_SKIP_RS"):
                nc.gpsimd.dma_start(out=rsout[:], in_=ptab[0:ROWS_PC, :])
            else:
                nc.gpsimd.collective_compute(
                    "ReduceScatter",
                    mybir.AluOpType.add,
                    replica_groups=[list(range(NCORES))],
                    ins=[ptab.opt()],
                    outs=[rsout.opt()],
                )

            # ---- tail ----------------------------------------------------
            rs_s = bigpool.tile([128, PB, TCOLS], f16, tag="rs")
            rsv = rsout.rearrange("(a p) d -> p a d", p=128)
            for g0 in range(0, PB, 5):
                nc.sync.dma_start(out=rs_s[:, g0:g0 + 5, :],
                                  in_=rsv[:, g0:g0 + 5, :])
            featp_s = cpool.tile([128, PB, 2 * D], f16, tag="featp")
            nc.sync.dma_start(out=featp_s[:], in_=featpD)
            indegp_s = cpool.tile([128, PB, 2], f16, tag="indegp")
            nc.sync.dma_start(out=indegp_s[:], in_=indegpD)

            idf = cpool.tile([128, PB, 2], f32, tag="idf")
            nc.vector.tensor_copy(idf[:], indegp_s[:])
            maskp = cpool.tile([128, PB, 2], f32, tag="maskp")
            nc.vector.tensor_scalar(maskp[:], idf[:], 0.0, None, OP.is_gt)
            cjp = cpool.tile([128, PB, 2], f32, tag="cjp")
            nc.vector.tensor_scalar_max(cjp[:], idf[:], 1.0)
            nc.scalar.activation(cjp[:], cjp[:], AF.Sqrt)
            nc.vector.reciprocal(cjp[:], cjp[:])
            nc.vector.tensor_mul(cjp[:], cjp[:], maskp[:])
            im1 = cpool.tile([128, PB, 2], f32, tag="im1")
            nc.vector.tensor_scalar(im1[:], maskp[:], -1.0, 1.0,
                                    OP.mult, OP.add)

            # h = agg * cj + featp * (1 - mask), in 5-block chunks so the
            # transposes start while later chunks still blend
            hp = bigpool.tile([128, PB, 2 * D], f32, tag="hp")
            tmp = bigpool.tile([128, PB, 2 * D], f32, tag="tmp")
            hT2 = bigpool.tile([2 * D + 1, ROWS_PC], f16, tag="hT2")
            nc.vector.memset(hT2[:, :], 1.0)
            CB = 5
            with tc.tile_pool(name="tp", bufs=4, space="PSUM") as tppool, \
                 tc.tile_pool(name="oc", bufs=3) as ocpool:
                for g0 in range(0, PB, CB):
                    bs = slice(g0, g0 + CB)
                    for par, c0 in ((0, 0), (1, 64)):
                        sl = slice(par * D, par * D + D)
                        nc.vector.tensor_tensor(
                            hp[:, bs, sl], rs_s[:, bs, c0:c0 + D],
                            cjp[:, bs, par:par + 1].to_broadcast(
                                [128, CB, D]), OP.mult)
                        nc.vector.tensor_tensor(
                            tmp[:, bs, sl], featp_s[:, bs, sl],
                            im1[:, bs, par:par + 1].to_broadcast(
                                [128, CB, D]), OP.mult)
                    nc.vector.tensor_add(hp[:, bs, :], hp[:, bs, :],
                                         tmp[:, bs, :])
                    for b in range(g0, g0 + CB):
                        tp = tppool.tile([2 * D, 128], f32, tag="tp")
                        nc.tensor.transpose(tp[:], hp[:, b, :], ident[:])
                        nc.vector.tensor_copy(
                            hT2[0:2 * D, b * 128:(b + 1) * 128], tp[:])

                CH = 512
                nch = (ROWS_PC + CH - 1) // CH
                for i in range(nch):
                    lo = i * CH
                    hi = min(lo + CH, ROWS_PC)
                    po = tppool.tile([2 * D, CH], f32, tag="po")
                    nc.tensor.matmul(po[:, 0:hi - lo], lhsT=wb2_s[:],
                                     rhs=hT2[:, lo:hi], start=True, stop=True)
                    oc = ocpool.tile([2 * D, CH], f32, tag="oc")
                    nc.scalar.activation(oc[:, 0:hi - lo], po[:, 0:hi - lo],
                                         AF.Relu)
                    nc.sync.dma_start(out=outD[:, lo:hi], in_=oc[:, 0:hi - lo])

    nc.compile()
    return nc


# ---------------------------------------------------------------------------

def kernel(features, src, dst, W, b):
    from concourse.bass_utils import run_bass_kernel_spmd

    in_maps, meta = _host_prep(features, src, dst)

    key = tuple(sorted(meta.items()))
    if key not in _CACHE:
        _CACHE[key] = _build_program(meta)
    nc = _CACHE[key]

    Wt = np.asarray(W, np.float32).T          # [48 in, 48 out]
    bv = np.asarray(b, np.float32)
    wb2 = np.zeros((2 * D + 1, 2 * D), np.float16)
    wb2[0:D, 0:D] = Wt
    wb2[D:2 * D, D:2 * D] = Wt
    wb2[2 * D, 0:D] = bv
    wb2[2 * D, D:2 * D] = bv
    ident = np.eye(128, dtype=np.float32)
    for m in in_maps:
        m["wb2"] = wb2
        m["ident"] = ident

    res = run_bass_kernel_spmd(nc, in_maps, core_ids=list(range(NCORES)))
    globals()["LAST_RESULTS"] = res

    out = np.empty((N, D), np.float32)
    for c in range(NCORES):
        o = res.results[c]["out"].astype(np.float32)
        nlo = c * NPC
        out[nlo:nlo + NPC:2] = o[0:D, :PAIRS_L].T
        out[nlo + 1:nlo + NPC:2] = o[D:2 * D, :PAIRS_L].T
    return np.ascontiguousarray(out, dtype=np.float32)
